# revision 22
# baseline (speedup 1.0000x reference)
"""CrossMambaFusion Trainium2 kernel — 8-core SPMD via bass/Tile. v2.

Sharding (hardcoded for B=2, C=256, H=W=64, Di=512, N=16, R=32, K=4):
  core c -> batch b = c//4, d_inner slice q = c%4 (128 channels).
  Feature-major [features, tokens] on device; (B,C,H,W) -> (C, L=4096).

v2 (vs v1 baseline):
  - L processed in two halves: the DVE selective scan of half 0 overlaps the
    front (proj/conv/dt) of half 1 on PE/ACT/Pool, and the AllToAll + tail of
    half 0 overlap the scan of half 1.
  - causal depthwise conv on PE as 4 shift-accumulated diagonal matmuls
    (removes ~86us of DVE stt chains).
  - softplus as 2 ACT ops: ln(1 + exp(x)) (x is small here; no overflow).
  - front/tail elementwise moved to Pool (gpsimd) so DVE does only ph1 stt,
    dtu and the scan, and the tail can run while DVE scans half 1.
  - per-half AllToAll [1024, 512]; token ownership interleaved so each core's
    tail input arrives right after its half's collective.

Token ownership: core (b, q) owns tokens {h*2048 + q*512 + [0,512) : h in 0,1}.
AllToAll in-block j = own y chunk of tokens [(j%4)*512, ...) of that half;
rows duplicated across batch groups; m_out weight rows of the other batch
group are zeroed (as in v1).
"""
import numpy as np
import ml_dtypes

bf16 = ml_dtypes.bfloat16

B, C, Hh, Ww = 2, 256, 64, 64
L = Hh * Ww
Di, N, R, KC = 512, 16, 32, 4
DQ = 128
NCORES = 8
LH = L // 2          # 2048
LC = 512             # front chunk
NLH = LH // LC       # 4 chunks per half
TQ = 512             # tokens per (core, half) piece

_cache = {}


def _build():
    import concourse.bass as bass
    import concourse.mybir as mybir
    import concourse.tile as tile
    from concourse import bacc

    fp32 = mybir.dt.float32
    bfl = mybir.dt.bfloat16
    AF = mybir.ActivationFunctionType
    OP = mybir.AluOpType
    ts = bass.ts

    nc = bacc.Bacc("TRN2", target_bir_lowering=False, num_devices=NCORES)

    def din(name, shape, dt=fp32):
        return nc.declare_dram_parameter(name, list(shape), dt, isOutput=False)

    dec_bf = din("dec_bf", (C, L), bfl)
    enc_bf = din("enc_bf", (C, L), bfl)
    dec_f32q = din("dec_f32q", (C, 2 * TQ), fp32)
    w_dec_x = din("w_dec_x", (C, Di), bfl)
    w_dec_g = din("w_dec_g", (C, Di), bfl)
    b_dec_x = din("b_dec_x", (Di, 1))
    b_dec_g = din("b_dec_g", (Di, 1))
    w_enc = din("w_enc", (C, Di), bfl)
    b_enc = din("b_enc", (Di, 1))
    w_in_x = din("w_in_x", (Di, DQ), bfl)      # own-slice columns
    b_in_x = din("b_in_x", (DQ, 1))            # own slice
    w_in_z = din("w_in_z", (Di, DQ), bfl)
    b_in_z = din("b_in_z", (DQ, 1))
    w_cd = din("w_cd", (DQ, KC, DQ), bfl)      # diagonal conv weights per k
    conv_b = din("conv_b", (DQ, 1))            # own slice
    w_xp = din("w_xp", (DQ, 128), bfl)         # own rows, cols zero-padded per batch
    w_dt = din("w_dt", (128, DQ), bfl)  # rows zero-padded per batch
    b_dt = din("b_dt", (DQ, 1))
    a_sl = din("a_sl", (DQ, N))
    sel_bc = din("sel_bc", (128, 2 * N, 128), bfl)  # one-hot B/C row selectors
    d_col = din("d_col", (DQ, 1))
    w_mo = din("w_mo", (2 * Di, Di), bfl)      # rows of other batch group zeroed
    b_mo = din("b_mo", (Di, 1))
    w_out = din("w_out", (Di, C), bfl)
    b_out = din("b_out", (C, 1))
    g_col = din("g_col", (C, 1))
    bln_col = din("bln_col", (C, 1))

    res_out = nc.declare_dram_parameter("res", [C, 2 * TQ], fp32, isOutput=True)

    with tile.TileContext(nc) as tc:
        import contextlib
        with contextlib.ExitStack() as stack:
            wpool = stack.enter_context(tc.tile_pool(name="weights", bufs=1))
            cpool = stack.enter_context(tc.tile_pool(name="consts", bufs=1))
            dpool = stack.enter_context(tc.tile_pool(name="drambuf", bufs=1, space="DRAM"))

            ar_in_h = [dpool.tile([128, LH], bfl, name=f"arin{j}") for j in range(2)]
            ar_out_h = [dpool.tile([128, LH], bfl, name=f"arout{j}") for j in range(2)]
            ln_mu = [dpool.tile([1, TQ], bfl, name=f"lnmu{h}") for h in range(2)]
            ln_rs = [dpool.tile([1, TQ], fp32, name=f"lnrs{h}") for h in range(2)]
            a2a_in = [dpool.tile([2 * Di, TQ], bfl, name=f"a2ai{h}") for h in range(2)]
            a2a_out = [dpool.tile([2 * Di, TQ], bfl, name=f"a2ao{h}") for h in range(2)]

            def wload(ap, kt, m, name):
                t = wpool.tile([128, kt, m], bfl, tag=name, name=name)
                nc.sync.dma_start(out=t[:], in_=ap.ap().rearrange("(t k) m -> k t m", k=128))
                return t

            sw_dec_x = wload(w_dec_x, 2, Di, "w_dec_x")
            sw_enc = wload(w_enc, 2, Di, "w_enc")
            sw_dec_g = wload(w_dec_g, 2, Di, "w_dec_g")
            sw_in_x = wload(w_in_x, 4, DQ, "w_in_x")
            sw_in_z = wload(w_in_z, 4, DQ, "w_in_z")
            sw_xp = wpool.tile([128, 128], bfl)
            nc.sync.dma_start(out=sw_xp[:], in_=w_xp.ap())
            sw_mo = wload(w_mo, 8, Di, "w_mo")
            sw_out = wload(w_out, 4, C, "w_out")
            sw_dt = wpool.tile([128, DQ], bfl)
            nc.sync.dma_start(out=sw_dt[:], in_=w_dt.ap())
            sw_sel = wpool.tile([128, 2 * N, 128], bfl)
            nc.sync.dma_start(out=sw_sel[:], in_=sel_bc.ap())
            sw_cd = wpool.tile([128, KC, DQ], bfl)
            nc.sync.dma_start(out=sw_cd[:], in_=w_cd.ap())

            def cload(ap, nt, name, cols=1):
                if nt == 1:
                    t = cpool.tile([128, cols], fp32, tag=name, name=name)
                    nc.sync.dma_start(out=t[:], in_=ap.ap())
                else:
                    t = cpool.tile([128, nt, cols], fp32, tag=name, name=name)
                    nc.sync.dma_start(out=t[:], in_=ap.ap().rearrange("(t k) o -> k t o", k=128))
                return t

            sb_dec_x = cload(b_dec_x, 4, "b_dec_x")
            sb_dec_g = cload(b_dec_g, 4, "b_dec_g")
            sb_enc = cload(b_enc, 4, "b_enc")
            sb_in_x = cload(b_in_x, 1, "b_in_x")
            sb_in_z = cload(b_in_z, 1, "b_in_z")
            s_convb = cload(conv_b, 1, "conv_b")
            sb_dt = cload(b_dt, 1, "b_dt")
            s_a = cload(a_sl, 1, "a_sl", cols=N)
            s_d = cload(d_col, 1, "d_col")
            sb_mo = cload(b_mo, 4, "b_mo")
            sb_out = cload(b_out, 2, "b_out")
            s_g = cload(g_col, 2, "g_col")
            s_bln = cload(bln_col, 2, "bln_col")

            # batch-select scalar -> sync-engine register for cond DMAs
            bsel = nc.declare_dram_parameter("bsel", [1, 1], mybir.dt.int32,
                                             isOutput=False)
            # persistent tiles
            ppool = stack.enter_context(tc.tile_pool(name="persist", bufs=1))
            s_dt = ppool.tile([128, L], bfl)
            s_siluz = ppool.tile([128, L], bfl)
            s_uown = ppool.tile([128, L], bfl)
            s_sgate = ppool.tile([128, 4, 2 * TQ], bfl)
            s_decf = ppool.tile([128, 2, 2 * TQ], fp32)
            carry = ppool.tile([128, N], fp32)
            nc.vector.memset(carry[:], 0.0)
            ones = ppool.tile([128, 1], fp32)
            nc.vector.memset(ones[:], 1.0)
            ones_bf = ppool.tile([128, 1], bfl)
            nc.vector.memset(ones_bf[:], 1.0)
            eps = ppool.tile([1, 1], fp32)
            nc.vector.memset(eps[:], 1e-5)
            s_bsel = ppool.tile([1, 1], mybir.dt.int32)
            nc.sync.dma_start(out=s_bsel[:], in_=bsel.ap())
            breg = nc.sync.alloc_register("bsel_reg")
            nc.sync.reg_load(breg, s_bsel[0:1, 0:1])
            bsnap = nc.sync.snap(breg, min_val=0, max_val=1)
            zreg = nc.sync.alloc_register("zero_reg")
            nc.sync.reg_mov(zreg, 0)
            zsnap = nc.sync.snap(zreg, min_val=0, max_val=0)
            cond_b0 = bsnap != zsnap   # true on batch-0 cores (bsel=1)
            cond_b1 = bsnap == zsnap

            # rotating pools (shared across phases)
            fpool = stack.enter_context(tc.tile_pool(name="front", bufs=2))
            f1c = stack.enter_context(tc.tile_pool(name="small", bufs=2))
            psA = stack.enter_context(tc.tile_pool(name="psA", bufs=6, space="PSUM"))
            psts = stack.enter_context(tc.tile_pool(name="psts", bufs=1, space="PSUM"))

            # ---- pretail: decoder gate on own tokens (sigmoid table) ----
            nc.sync.dma_start(out=s_decf[:],
                              in_=dec_f32q.ap().rearrange("(t k) l -> k t l", k=128))
            with tc.tile_pool(name="pret", bufs=1) as prepool:
                s_decq = prepool.tile([128, 2, 2 * TQ], bfl)
                nc.gpsimd.tensor_copy(s_decq[:], s_decf[:])
                for lc in range(2):
                    ls = ts(lc, TQ)
                    for m in range(4):
                        ps_g = psA.tile([128, TQ], fp32, tag="mm", name="ps_g")
                        for t in range(2):
                            nc.tensor.matmul(ps_g[:], sw_dec_g[:, t, ts(m, 128)],
                                             s_decq[:, t, ls], start=(t == 0), stop=(t == 1))
                        nc.scalar.activation(s_sgate[:, m, ls], ps_g[:], AF.Sigmoid,
                                             bias=sb_dec_g[:, m, :])

            dec_r = dec_bf.ap().rearrange("(t k) l -> k t l", k=128)
            enc_r = enc_bf.ap().rearrange("(t k) l -> k t l", k=128)
            fstate = {}
            G8 = [[0, 1, 2, 3, 4, 5, 6, 7]]
            BROW = 32   # x_dbl rows: [0:32)=dt_in, [32:48)=B, [48:64)=C
            s_dtraw = ppool.tile([128, LH], bfl)   # staged softplus input (1 half)
            s_ex = ppool.tile([128, LH], bfl)      # softplus exp scratch

            def ph1_chunk(lc, on_pool=False):
                # combined = dec_x*sig(enc_p) + enc_p for one chunk
                ls = ts(lc, LC)
                s_dec = fpool.tile([128, 2, LC], bfl, tag="s_dec", name="s_dec")
                s_enc = fpool.tile([128, 2, LC], bfl, tag="s_enc", name="s_enc")
                nc.sync.dma_start(out=s_dec[:], in_=dec_r[:, :, ls])
                nc.sync.dma_start(out=s_enc[:], in_=enc_r[:, :, ls])
                comb = fpool.tile([128, 4, LC], bfl, tag="comb", name="comb")
                for m in range(4):
                    ps_dx = psA.tile([128, LC], fp32, tag="mm", name="ps_dx")
                    ps_ep = psA.tile([128, LC], fp32, tag="mm", name="ps_ep")
                    for t in range(2):
                        nc.tensor.matmul(ps_dx[:], sw_dec_x[:, t, ts(m, 128)],
                                         s_dec[:, t, :], start=(t == 0), stop=(t == 1))
                    for t in range(2):
                        nc.tensor.matmul(ps_ep[:], sw_enc[:, t, ts(m, 128)],
                                         s_enc[:, t, :], start=(t == 0), stop=(t == 1))
                    sg = f1c.tile([128, LC], bfl, tag="sg", name="sg")
                    nc.scalar.activation(sg[:], ps_ep[:], AF.Sigmoid,
                                         bias=sb_enc[:, m, :])
                    if on_pool:
                        # keep DVE free during the scan: materialize biased
                        # dx/ep via ACT (table-free), combine on Pool
                        dxs = f1c.tile([128, LC], bfl, tag="dxs", name="dxs")
                        nc.scalar.activation(dxs[:], ps_dx[:], AF.Identity,
                                             bias=sb_dec_x[:, m, :])
                        nc.scalar.activation(comb[:, m, :], ps_ep[:], AF.Identity,
                                             bias=sb_enc[:, m, :])
                        tm = f1c.tile([128, LC], bfl, tag="tm", name="tm")
                        nc.gpsimd.tensor_tensor(tm[:], dxs[:], sg[:], OP.mult)
                        nc.gpsimd.tensor_tensor(comb[:, m, :], comb[:, m, :], tm[:],
                                                OP.add)
                    else:
                        tm = f1c.tile([128, LC], bfl, tag="tm", name="tm")
                        nc.vector.scalar_tensor_tensor(tm[:], ps_dx[:],
                                                       sb_dec_x[:, m, :], sg[:],
                                                       OP.add, OP.mult)
                        nc.vector.scalar_tensor_tensor(comb[:, m, :], ps_ep[:],
                                                       sb_enc[:, m, :], tm[:],
                                                       OP.add, OP.add)
                return comb

            def front_rest_chunk(lc, comb, act_bias):
                # in_proj (own slice), conv (PE diag), padded x_proj partial,
                # stage partial x_dbl for the per-quarter AllReduce.
                ls = ts(lc, LC)
                half, i = lc // NLH, lc % NLH
                xm = fpool.tile([128, 3 + LC], bfl, tag="xm", name="xm")
                if lc == 0:
                    nc.gpsimd.memset(xm[:, 0:3], 0.0)
                else:
                    nc.scalar.activation(xm[:, 0:3],
                                         fstate["xm_prev"][:, LC:LC + 3],
                                         AF.Identity)
                ps_xm = psA.tile([128, LC], fp32, tag="mm", name="ps_xm")
                for t in range(4):
                    nc.tensor.matmul(ps_xm[:], sw_in_x[:, t, :],
                                     comb[:, t, :], start=(t == 0), stop=(t == 3))
                if act_bias:
                    nc.scalar.activation(xm[:, 3:3 + LC], ps_xm[:],
                                         AF.Identity, bias=sb_in_x[:, 0:1])
                else:
                    nc.vector.tensor_scalar(xm[:, 3:3 + LC], ps_xm[:],
                                            sb_in_x[:, 0:1], None, OP.add)
                ps_z = psA.tile([128, LC], fp32, tag="mm", name="ps_z")
                for t in range(4):
                    nc.tensor.matmul(ps_z[:], sw_in_z[:, t, :], comb[:, t, :],
                                     start=(t == 0), stop=(t == 3))
                nc.scalar.activation(s_siluz[:, ls], ps_z[:], AF.Silu,
                                     bias=sb_in_z[:, 0:1])
                # conv on PE: 4 shift-accumulated diagonal matmuls (own slice)
                ps_c = psA.tile([128, LC], fp32, tag="mm", name="ps_c")
                for k in range(KC):
                    nc.tensor.matmul(ps_c[:], sw_cd[:, k, :], xm[:, k:k + LC],
                                     start=(k == 0), stop=(k == KC - 1))
                nc.scalar.activation(s_uown[:, ls], ps_c[:], AF.Silu,
                                     bias=s_convb[:, 0:1])
                fstate["xm_prev"] = xm
                # padded x_proj partial: out rows [b*64, b*64+64) hold x_dbl
                ps_xd = psA.tile([128, LC], fp32, tag="mm", name="ps_xd")
                nc.tensor.matmul(ps_xd[:], sw_xp[:, :], s_uown[:, ls],
                                 start=True, stop=True)
                ar_st = f1c.tile([128, LC], bfl, tag="ar_st", name="ar_st")
                nc.scalar.activation(ar_st[:], ps_xd[:], AF.Copy)
                nc.sync.dma_start(out=ar_in_h[half][:, ts(i, LC)], in_=ar_st[:])
                if i == NLH - 1:
                    nc.gpsimd.collective_compute(
                        "AllReduce", OP.add, replica_groups=G8,
                        ins=[ar_in_h[half][:, :]], outs=[ar_out_h[half][:, :]],
                    )

            def softplus_half(half):
                # dt matmul over all 128 AllReduced rows (other batch rows are
                # zeroed in sw_dt), then softplus(x) = ln(1 + exp(x + dt_b))
                hs = ts(half, LH)
                dtin = f1c.tile([128, LH], bfl, tag="dtin", name="dtin")
                nc.sync.dma_start(out=dtin[:], in_=ar_out_h[half][:, :])
                fstate[f"xdbl{half}"] = dtin
                for fc in range(4):
                    ps_dt = psA.tile([128, LC], fp32, tag="mm", name="ps_dt")
                    nc.tensor.matmul(ps_dt[:], sw_dt[:, :], dtin[:, ts(fc, LC)],
                                     start=True, stop=True)
                    nc.scalar.activation(s_dtraw[:, ts(fc, LC)], ps_dt[:], AF.Copy)
                nc.scalar.activation(s_ex[:], s_dtraw[:], AF.Exp, bias=sb_dt[:, 0:1])
                nc.scalar.activation(s_dt[:, hs], s_ex[:], AF.Ln, bias=1.0)

            def front(half):
                for i in range(NLH):
                    lc = half * NLH + i
                    comb = ph1_chunk(lc)
                    front_rest_chunk(lc, comb, act_bias=False)
                softplus_half(half)

            spool = stack.enter_context(tc.tile_pool(name="scan", bufs=1))
            sbc = stack.enter_context(tc.tile_pool(name="scanbc", bufs=2))
            sy = stack.enter_context(tc.tile_pool(name="scany", bufs=1))

            def scan(half, fillers=None):
                fillers = fillers or {}
                hs = ts(half, LH)
                s_dtu = sy.tile([128, LH], bfl, tag="dtu", name="s_dtu")
                nc.vector.tensor_tensor(s_dtu[:], s_dt[:, hs], s_uown[:, hs], OP.mult)
                s_uD = sy.tile([128, LH], bfl, tag="uD", name="s_uD")
                nc.vector.tensor_scalar(s_uD[:], s_uown[:, hs], s_d[:, 0:1], None,
                                        OP.mult)
                ysum = sy.tile([128, LH], bfl, tag="ysum", name="ysum")
                xdbl = fstate[f"xdbl{half}"]
                for n in range(N):
                    w = n % 2
                    bc2 = sbc.tile([128, 2, LH], bfl, tag="bc2", name="bc2")
                    for s in range(2):          # 0: B row, 1: C row
                        for fc in range(4):
                            ps_bc = psA.tile([128, LC], fp32, tag="mm",
                                             name="ps_bc")
                            nc.tensor.matmul(ps_bc[:], sw_sel[:, s * N + n, :],
                                             xdbl[:, ts(fc, LC)],
                                             start=True, stop=True)
                            nc.scalar.activation(bc2[:, s, ts(fc, LC)], ps_bc[:],
                                                 AF.Copy)
                    bbc = bc2[:, 0, :]
                    cbc = bc2[:, 1, :]
                    a = spool.tile([128, LH], bfl, tag=f"a{n % 3}", name="a")
                    nc.scalar.activation(a[:], s_dt[:, hs], AF.Exp,
                                         scale=s_a[:, n:n + 1])
                    bt = spool.tile([128, LH], bfl, tag=f"b{w}", name=f"b{w}")
                    nc.vector.tensor_tensor(bt[:], s_dtu[:], bbc, OP.mult)
                    h = spool.tile([128, LH], bfl, tag=f"h{w}", name=f"h{w}")
                    nc.vector.tensor_tensor_scan(h[:], a[:], bt[:],
                                                 carry[:, n:n + 1], OP.mult, OP.add)
                    if half == 0:
                        nc.vector.tensor_copy(carry[:, n:n + 1], h[:, LH - 1:LH])
                    nc.vector.tensor_tensor(h[:], h[:], cbc, OP.mult)
                    if n == 0:
                        nc.vector.tensor_tensor(ysum[:], h[:], s_uD[:], OP.add)
                    else:
                        nc.vector.tensor_tensor(ysum[:], ysum[:], h[:], OP.add)
                    if n in fillers:
                        fillers[n]()
                y_h = sy.tile([128, LH], bfl, tag="y", name="y_h")
                nc.vector.tensor_tensor(y_h[:], ysum[:], s_siluz[:, hs], OP.mult)
                for j in range(8):
                    nc.sync.dma_start(out=a2a_in[half][j * 128:(j + 1) * 128, :],
                                      in_=y_h[:, ts(j % 4, TQ)])
                nc.gpsimd.collective_compute(
                    "AllToAll", mybir.AluOpType.bypass, replica_groups=G8,
                    ins=[a2a_in[half][:, :]], outs=[a2a_out[half][:, :]],
                )

            tpool = stack.enter_context(tc.tile_pool(name="tail", bufs=1))

            tstate = {}

            def tail_main(half):
                hs = ts(half, TQ)
                s_yall = tpool.tile([128, 8, TQ], bfl, tag="yall", name="s_yall")
                a2a_r = a2a_out[half][:, :].rearrange("(t k) l -> k t l", k=128)
                nc.sync.dma_start(out=s_yall[:], in_=a2a_r)
                s_gated = tpool.tile([128, 4, TQ], bfl, tag="gated", name="s_gated")
                for m in range(4):
                    ps_mo = psA.tile([128, TQ], fp32, tag="mm", name="ps_mo")
                    for t in range(8):
                        nc.tensor.matmul(ps_mo[:], sw_mo[:, t, ts(m, 128)],
                                         s_yall[:, t, :], start=(t == 0), stop=(t == 7))
                    spr = f1c.tile([128, TQ], bfl, tag="spr", name="spr")
                    nc.scalar.activation(spr[:], ps_mo[:], AF.Identity,
                                         bias=sb_mo[:, m, :])
                    nc.gpsimd.tensor_tensor(s_gated[:, m, :], spr[:],
                                            s_sgate[:, m, hs], OP.mult)
                s_res = tpool.tile([128, 2, TQ], bfl, tag=f"res{half}", name="s_res")
                ps_sum = psts.tile([1, TQ], fp32, tag="ps_sum", name="ps_sum")
                ps_sq = psts.tile([1, TQ], fp32, tag="ps_sq", name="ps_sq")
                for m in range(2):
                    ps_o = psA.tile([128, TQ], fp32, tag="mm", name="ps_o")
                    for t in range(4):
                        nc.tensor.matmul(ps_o[:], sw_out[:, t, ts(m, 128)],
                                         s_gated[:, t, :], start=(t == 0), stop=(t == 3))
                    otmp = f1c.tile([128, TQ], bfl, tag="otmp", name="otmp")
                    nc.scalar.activation(otmp[:], ps_o[:], AF.Identity,
                                         bias=sb_out[:, m, :])
                    nc.gpsimd.tensor_tensor(s_res[:, m, :], otmp[:],
                                            s_decf[:, m, hs], OP.add)
                    res2 = f1c.tile([128, TQ], bfl, tag="res2", name="res2")
                    nc.scalar.activation(res2[:], s_res[:, m, :], AF.Square)
                    nc.tensor.matmul(ps_sum[:], ones_bf[:], s_res[:, m, :],
                                     start=(m == 0), stop=(m == 1))
                    nc.tensor.matmul(ps_sq[:], ones_bf[:], res2[:],
                                     start=(m == 0), stop=(m == 1))
                mu = tpool.tile([1, TQ], bfl, tag="mu", name="mu")
                nc.scalar.activation(mu[:], ps_sum[:], AF.Copy, scale=1.0 / C)
                musq = tpool.tile([1, TQ], bfl, tag="musq", name="musq")
                nc.scalar.activation(musq[:], mu[:], AF.Square)
                sq_sb = tpool.tile([1, TQ], bfl, tag="sq_sb", name="sq_sb")
                nc.scalar.activation(sq_sb[:], ps_sq[:], AF.Copy, scale=1.0 / C)
                var = tpool.tile([1, TQ], bfl, tag="var", name="var")
                nc.gpsimd.tensor_tensor(var[:], sq_sb[:], musq[:], OP.subtract)
                sd = tpool.tile([1, TQ], fp32, tag=f"sd{half}", name="sd")
                nc.scalar.activation(sd[:], var[:], AF.Sqrt, bias=eps[:, 0:1])
                nc.sync.dma_start(out=ln_mu[half][0:1, :], in_=mu[:])
                tstate[half] = (s_res, sd)

            def tail_fin(half):
                hs = ts(half, TQ)
                s_res, sd = tstate[half]
                rstd = tpool.tile([1, TQ], fp32, tag="rstd", name="rstd")
                nc.vector.reciprocal(rstd[:], sd[:])
                nc.sync.dma_start(out=ln_rs[half][0:1, :], in_=rstd[:])
                mu_bc = tpool.tile([128, TQ], bfl, tag="mu_bc", name="mu_bc")
                rs_bc = tpool.tile([128, TQ], fp32, tag="rs_bc", name="rs_bc")
                lnm = ln_mu[half][0:1, 0:1]
                lnr = ln_rs[half][0:1, 0:1]
                nc.sync.dma_start(out=mu_bc[:], in_=bass.AP(
                    tensor=lnm.tensor, offset=lnm.offset, ap=[[0, 128], [1, TQ]]))
                nc.sync.dma_start(out=rs_bc[:], in_=bass.AP(
                    tensor=lnr.tensor, offset=lnr.offset, ap=[[0, 128], [1, TQ]]))
                for m in range(2):
                    nc.gpsimd.tensor_tensor(s_res[:, m, :], s_res[:, m, :], mu_bc[:],
                                            OP.subtract)
                    nc.gpsimd.tensor_tensor(s_res[:, m, :], s_res[:, m, :], rs_bc[:],
                                            OP.mult)
                    t1 = tpool.tile([128, TQ], fp32, tag="t1", name="t1")
                    nc.scalar.activation(t1[:], s_res[:, m, :], AF.Identity,
                                         scale=s_g[:, m, :], bias=s_bln[:, m, :])
                    nc.sync.dma_start(
                        out=res_out.ap().rearrange("(t k) l -> k t l", k=128)[:, m, hs],
                        in_=t1[:])

            # ---- emission order ----
            front(0)
            front(1)
            scan(0)
            scan(1, {14: lambda: tail_main(0)})
            tail_fin(0)
            tail_main(1)
            tail_fin(1)

    nc.compile()
    return nc


def _in_maps(inp):
    A = -np.exp(inp["A_log"].astype(np.float32))
    dec_T = inp["decoder_feat"].reshape(B, C, L)
    enc_T = inp["encoder_feat"].reshape(B, C, L)
    dec_T_bf = dec_T.astype(bf16)
    enc_T_bf = enc_T.astype(bf16)

    def col(x):
        return np.ascontiguousarray(np.asarray(x, np.float32).reshape(-1, 1))

    common = {
        "w_dec_x": np.ascontiguousarray(inp["dec_w"][:, :Di].astype(bf16)),
        "w_dec_g": np.ascontiguousarray(inp["dec_w"][:, Di:].astype(bf16)),
        "b_dec_x": col(inp["dec_b"][:Di]),
        "b_dec_g": col(inp["dec_b"][Di:]),
        "w_enc": inp["enc_w"].astype(bf16),
        "b_enc": col(inp["enc_b"]),
        "b_mo": col(inp["m_out_b"]),
        "w_out": inp["out_w"].astype(bf16),
        "b_out": col(inp["out_b"]),
        "g_col": col(inp["ln_g"]),
        "bln_col": col(inp["ln_b"]),
    }

    in_maps = []
    for c in range(NCORES):
        b, q = c // 4, c % 4
        ds = slice(q * DQ, (q + 1) * DQ)
        m = dict(common)
        m["dec_bf"] = dec_T_bf[b]
        m["enc_bf"] = enc_T_bf[b]
        tok = np.r_[np.arange(q * TQ, (q + 1) * TQ),
                    np.arange(LH + q * TQ, LH + (q + 1) * TQ)]
        m["dec_f32q"] = np.ascontiguousarray(dec_T[b][:, tok].astype(np.float32))
        m["w_in_x"] = np.ascontiguousarray(inp["in_w"][:, :Di][:, ds].astype(bf16))
        m["b_in_x"] = col(inp["in_b"][:Di][ds])
        m["w_in_z"] = np.ascontiguousarray(
            inp["in_w"][:, Di + q * DQ:Di + (q + 1) * DQ].astype(bf16))
        m["b_in_z"] = col(inp["in_b"][Di + q * DQ:Di + (q + 1) * DQ])
        cw = inp["conv_w"][ds, 0, :].astype(np.float32)     # (DQ, KC) own slice
        wcd = np.zeros((DQ, KC, DQ), np.float32)
        idx = np.arange(DQ)
        for k in range(KC):
            wcd[idx, k, idx] = cw[idx, k]
        m["w_cd"] = wcd.astype(bf16)
        m["conv_b"] = col(inp["conv_b"][ds])
        # padded x_proj: own 128 rows -> 128 out cols, my batch block only
        wxp = np.zeros((DQ, 128), np.float32)
        wxp[:, b * 64:b * 64 + 2 * R] = inp["x_proj_w"][ds, :]
        m["w_xp"] = wxp.astype(bf16)
        # padded dt weights: contraction over all 128 AR rows, other rows 0
        wdt = np.zeros((128, DQ), np.float32)
        wdt[b * 64:b * 64 + R, :] = inp["dt_w"][:, ds]
        m["w_dt"] = wdt.astype(bf16)
        m["b_dt"] = col(inp["dt_b"][ds])
        m["bsel"] = np.array([[1 - b]], np.int32)
        sel = np.zeros((128, 2 * N, 128), np.float32)
        for n in range(N):
            sel[b * 64 + R + n, n, :] = 1.0          # B row
            sel[b * 64 + R + N + n, N + n, :] = 1.0  # C row
        m["sel_bc"] = sel.astype(bf16)
        wmo8 = np.zeros((2 * Di, Di), np.float32)
        for r in range(8):
            if r // 4 == b:
                rq = r % 4
                wmo8[r * DQ:(r + 1) * DQ] = inp["m_out_w"][rq * DQ:(rq + 1) * DQ]
        m["w_mo"] = wmo8.astype(bf16)
        m["a_sl"] = np.ascontiguousarray(A[ds])
        m["d_col"] = col(inp["D_param"][ds])
        in_maps.append(m)
    return in_maps


def kernel(**inputs):
    from concourse.bass_utils import run_bass_kernel_spmd

    inp = {k: np.asarray(v) for k, v in inputs.items()}
    if "nc" not in _cache:
        _cache["nc"] = _build()
    res = run_bass_kernel_spmd(_cache["nc"], _in_maps(inp), list(range(NCORES)))
    out = np.zeros((B, C, L), np.float32)
    for c in range(NCORES):
        b, q = c // 4, c % 4
        r = res.results[c]["res"]
        out[b][:, q * TQ:(q + 1) * TQ] = r[:, 0:TQ]
        out[b][:, LH + q * TQ:LH + (q + 1) * TQ] = r[:, TQ:2 * TQ]
    return out.reshape(B, C, Hh, Ww)


def run_traced(inp):
    from concourse.bass_utils import run_bass_kernel_spmd

    if "nc" not in _cache:
        _cache["nc"] = _build()
    return run_bass_kernel_spmd(_cache["nc"], _in_maps(inp), list(range(NCORES)),
                                trace=True)


# revision 23
# speedup vs baseline: 1.0308x; 1.0308x over previous
"""CrossMambaFusion Trainium2 kernel — 8-core SPMD via bass/Tile. v2.

Sharding (hardcoded for B=2, C=256, H=W=64, Di=512, N=16, R=32, K=4):
  core c -> batch b = c//4, d_inner slice q = c%4 (128 channels).
  Feature-major [features, tokens] on device; (B,C,H,W) -> (C, L=4096).

v2 (vs v1 baseline):
  - L processed in two halves: the DVE selective scan of half 0 overlaps the
    front (proj/conv/dt) of half 1 on PE/ACT/Pool, and the AllToAll + tail of
    half 0 overlap the scan of half 1.
  - causal depthwise conv on PE as 4 shift-accumulated diagonal matmuls
    (removes ~86us of DVE stt chains).
  - softplus as 2 ACT ops: ln(1 + exp(x)) (x is small here; no overflow).
  - front/tail elementwise moved to Pool (gpsimd) so DVE does only ph1 stt,
    dtu and the scan, and the tail can run while DVE scans half 1.
  - per-half AllToAll [1024, 512]; token ownership interleaved so each core's
    tail input arrives right after its half's collective.

Token ownership: core (b, q) owns tokens {h*2048 + q*512 + [0,512) : h in 0,1}.
AllToAll in-block j = own y chunk of tokens [(j%4)*512, ...) of that half;
rows duplicated across batch groups; m_out weight rows of the other batch
group are zeroed (as in v1).
"""
import numpy as np
import ml_dtypes

bf16 = ml_dtypes.bfloat16

B, C, Hh, Ww = 2, 256, 64, 64
L = Hh * Ww
Di, N, R, KC = 512, 16, 32, 4
DQ = 128
NCORES = 8
LH = L // 2          # 2048
LC = 512             # front chunk
NLH = LH // LC       # 4 chunks per half
TQ = 512             # tokens per (core, half) piece

_cache = {}


def _build():
    import concourse.bass as bass
    import concourse.mybir as mybir
    import concourse.tile as tile
    from concourse import bacc

    fp32 = mybir.dt.float32
    bfl = mybir.dt.bfloat16
    AF = mybir.ActivationFunctionType
    OP = mybir.AluOpType
    ts = bass.ts

    nc = bacc.Bacc("TRN2", target_bir_lowering=False, num_devices=NCORES)

    def din(name, shape, dt=fp32):
        return nc.declare_dram_parameter(name, list(shape), dt, isOutput=False)

    dec_bf = din("dec_bf", (C, L), bfl)
    enc_bf = din("enc_bf", (C, L), bfl)
    dec_f32q = din("dec_f32q", (C, 2 * TQ), fp32)
    w_dec_x = din("w_dec_x", (C, Di), bfl)
    w_dec_g = din("w_dec_g", (C, Di), bfl)
    b_dec_x = din("b_dec_x", (Di, 1))
    b_dec_g = din("b_dec_g", (Di, 1))
    w_enc = din("w_enc", (C, Di), bfl)
    b_enc = din("b_enc", (Di, 1))
    w_in_x = din("w_in_x", (Di, DQ), bfl)      # own-slice columns
    b_in_x = din("b_in_x", (DQ, 1))            # own slice
    w_in_z = din("w_in_z", (Di, DQ), bfl)
    b_in_z = din("b_in_z", (DQ, 1))
    w_cd = din("w_cd", (DQ, KC, DQ), bfl)      # diagonal conv weights per k
    conv_b = din("conv_b", (DQ, 1))            # own slice
    w_xp = din("w_xp", (DQ, 128), bfl)         # own rows, cols zero-padded per batch
    w_dt = din("w_dt", (128, DQ), bfl)  # rows zero-padded per batch
    b_dt = din("b_dt", (DQ, 1))
    a_sl = din("a_sl", (DQ, N))
    sel_bc = din("sel_bc", (128, 2 * N, 128), bfl)  # one-hot B/C row selectors
    d_col = din("d_col", (DQ, 1))
    w_mo = din("w_mo", (2 * Di, Di), bfl)      # rows of other batch group zeroed
    b_mo = din("b_mo", (Di, 1))
    w_out = din("w_out", (Di, C), bfl)
    b_out = din("b_out", (C, 1))
    g_col = din("g_col", (C, 1))
    bln_col = din("bln_col", (C, 1))

    res_out = nc.declare_dram_parameter("res", [C, 2 * TQ], fp32, isOutput=True)

    with tile.TileContext(nc) as tc:
        import contextlib
        with contextlib.ExitStack() as stack:
            wpool = stack.enter_context(tc.tile_pool(name="weights", bufs=1))
            cpool = stack.enter_context(tc.tile_pool(name="consts", bufs=1))
            dpool = stack.enter_context(tc.tile_pool(name="drambuf", bufs=1, space="DRAM"))

            ar_in_h = [dpool.tile([128, LH], bfl, name=f"arin{j}") for j in range(2)]
            ar_out_h = [dpool.tile([128, LH], bfl, name=f"arout{j}") for j in range(2)]
            ln_mu = [dpool.tile([1, TQ], bfl, name=f"lnmu{h}") for h in range(2)]
            ln_rs = [dpool.tile([1, TQ], fp32, name=f"lnrs{h}") for h in range(2)]
            a2a_in = [dpool.tile([2 * Di, TQ], bfl, name=f"a2ai{h}") for h in range(2)]
            a2a_out = [dpool.tile([2 * Di, TQ], bfl, name=f"a2ao{h}") for h in range(2)]

            def wload(ap, kt, m, name):
                t = wpool.tile([128, kt, m], bfl, tag=name, name=name)
                nc.sync.dma_start(out=t[:], in_=ap.ap().rearrange("(t k) m -> k t m", k=128))
                return t

            sw_dec_x = wload(w_dec_x, 2, Di, "w_dec_x")
            sw_enc = wload(w_enc, 2, Di, "w_enc")
            sw_dec_g = wload(w_dec_g, 2, Di, "w_dec_g")
            sw_in_x = wload(w_in_x, 4, DQ, "w_in_x")
            sw_in_z = wload(w_in_z, 4, DQ, "w_in_z")
            sw_xp = wpool.tile([128, 128], bfl)
            nc.sync.dma_start(out=sw_xp[:], in_=w_xp.ap())
            sw_mo = wload(w_mo, 8, Di, "w_mo")
            sw_out = wload(w_out, 4, C, "w_out")
            sw_dt = wpool.tile([128, DQ], bfl)
            nc.sync.dma_start(out=sw_dt[:], in_=w_dt.ap())
            sw_sel = wpool.tile([128, 2 * N, 128], bfl)
            nc.sync.dma_start(out=sw_sel[:], in_=sel_bc.ap())
            sw_cd = wpool.tile([128, KC, DQ], bfl)
            nc.sync.dma_start(out=sw_cd[:], in_=w_cd.ap())

            def cload(ap, nt, name, cols=1):
                if nt == 1:
                    t = cpool.tile([128, cols], fp32, tag=name, name=name)
                    nc.sync.dma_start(out=t[:], in_=ap.ap())
                else:
                    t = cpool.tile([128, nt, cols], fp32, tag=name, name=name)
                    nc.sync.dma_start(out=t[:], in_=ap.ap().rearrange("(t k) o -> k t o", k=128))
                return t

            sb_dec_x = cload(b_dec_x, 4, "b_dec_x")
            sb_dec_g = cload(b_dec_g, 4, "b_dec_g")
            sb_enc = cload(b_enc, 4, "b_enc")
            sb_in_x = cload(b_in_x, 1, "b_in_x")
            sb_in_z = cload(b_in_z, 1, "b_in_z")
            s_convb = cload(conv_b, 1, "conv_b")
            sb_dt = cload(b_dt, 1, "b_dt")
            s_a = cload(a_sl, 1, "a_sl", cols=N)
            s_d = cload(d_col, 1, "d_col")
            sb_mo = cload(b_mo, 4, "b_mo")
            sb_out = cload(b_out, 2, "b_out")
            s_g = cload(g_col, 2, "g_col")
            s_bln = cload(bln_col, 2, "bln_col")

            # batch-select scalar -> sync-engine register for cond DMAs
            bsel = nc.declare_dram_parameter("bsel", [1, 1], mybir.dt.int32,
                                             isOutput=False)
            # persistent tiles
            ppool = stack.enter_context(tc.tile_pool(name="persist", bufs=1))
            s_dt = ppool.tile([128, L], bfl)
            s_siluz = ppool.tile([128, L], bfl)
            s_uown = ppool.tile([128, L], bfl)
            s_sgate = ppool.tile([128, 4, 2 * TQ], bfl)
            s_decf = ppool.tile([128, 2, 2 * TQ], fp32)
            carry = ppool.tile([128, N], fp32)
            nc.vector.memset(carry[:], 0.0)
            ones = ppool.tile([128, 1], fp32)
            nc.vector.memset(ones[:], 1.0)
            ones_bf = ppool.tile([128, 1], bfl)
            nc.vector.memset(ones_bf[:], 1.0)
            eps = ppool.tile([1, 1], fp32)
            nc.vector.memset(eps[:], 1e-5)
            s_bsel = ppool.tile([1, 1], mybir.dt.int32)
            nc.sync.dma_start(out=s_bsel[:], in_=bsel.ap())
            breg = nc.sync.alloc_register("bsel_reg")
            nc.sync.reg_load(breg, s_bsel[0:1, 0:1])
            bsnap = nc.sync.snap(breg, min_val=0, max_val=1)
            zreg = nc.sync.alloc_register("zero_reg")
            nc.sync.reg_mov(zreg, 0)
            zsnap = nc.sync.snap(zreg, min_val=0, max_val=0)
            cond_b0 = bsnap != zsnap   # true on batch-0 cores (bsel=1)
            cond_b1 = bsnap == zsnap

            # rotating pools (shared across phases)
            fpool = stack.enter_context(tc.tile_pool(name="front", bufs=2))
            f1c = stack.enter_context(tc.tile_pool(name="small", bufs=2))
            psA = stack.enter_context(tc.tile_pool(name="psA", bufs=6, space="PSUM"))
            psts = stack.enter_context(tc.tile_pool(name="psts", bufs=1, space="PSUM"))

            # ---- pretail: decoder gate on own tokens (sigmoid table) ----
            nc.sync.dma_start(out=s_decf[:],
                              in_=dec_f32q.ap().rearrange("(t k) l -> k t l", k=128))
            with tc.tile_pool(name="pret", bufs=1) as prepool:
                s_decq = prepool.tile([128, 2, 2 * TQ], bfl)
                nc.gpsimd.tensor_copy(s_decq[:], s_decf[:])
                for lc in range(2):
                    ls = ts(lc, TQ)
                    for m in range(4):
                        ps_g = psA.tile([128, TQ], fp32, tag="mm", name="ps_g")
                        for t in range(2):
                            nc.tensor.matmul(ps_g[:], sw_dec_g[:, t, ts(m, 128)],
                                             s_decq[:, t, ls], start=(t == 0), stop=(t == 1))
                        nc.scalar.activation(s_sgate[:, m, ls], ps_g[:], AF.Sigmoid,
                                             bias=sb_dec_g[:, m, :])

            dec_r = dec_bf.ap().rearrange("(t k) l -> k t l", k=128)
            enc_r = enc_bf.ap().rearrange("(t k) l -> k t l", k=128)
            fstate = {}
            G8 = [[0, 1, 2, 3, 4, 5, 6, 7]]
            BROW = 32   # x_dbl rows: [0:32)=dt_in, [32:48)=B, [48:64)=C
            s_dtraw = ppool.tile([128, LH], bfl)   # staged softplus input (1 half)
            s_ex = ppool.tile([128, LH], bfl)      # softplus exp scratch

            def ph1_chunk(lc, on_pool=False):
                # combined = dec_x*sig(enc_p) + enc_p for one chunk
                ls = ts(lc, LC)
                s_dec = fpool.tile([128, 2, LC], bfl, tag="s_dec", name="s_dec")
                s_enc = fpool.tile([128, 2, LC], bfl, tag="s_enc", name="s_enc")
                nc.sync.dma_start(out=s_dec[:], in_=dec_r[:, :, ls])
                nc.sync.dma_start(out=s_enc[:], in_=enc_r[:, :, ls])
                comb = fpool.tile([128, 4, LC], bfl, tag="comb", name="comb")
                for m in range(4):
                    ps_dx = psA.tile([128, LC], fp32, tag="mm", name="ps_dx")
                    ps_ep = psA.tile([128, LC], fp32, tag="mm", name="ps_ep")
                    for t in range(2):
                        nc.tensor.matmul(ps_dx[:], sw_dec_x[:, t, ts(m, 128)],
                                         s_dec[:, t, :], start=(t == 0), stop=(t == 1))
                    for t in range(2):
                        nc.tensor.matmul(ps_ep[:], sw_enc[:, t, ts(m, 128)],
                                         s_enc[:, t, :], start=(t == 0), stop=(t == 1))
                    sg = f1c.tile([128, LC], bfl, tag="sg", name="sg")
                    nc.scalar.activation(sg[:], ps_ep[:], AF.Sigmoid,
                                         bias=sb_enc[:, m, :])
                    if on_pool:
                        # keep DVE free during the scan: materialize biased
                        # dx/ep via ACT (table-free), combine on Pool
                        dxs = f1c.tile([128, LC], bfl, tag="dxs", name="dxs")
                        nc.scalar.activation(dxs[:], ps_dx[:], AF.Identity,
                                             bias=sb_dec_x[:, m, :])
                        nc.scalar.activation(comb[:, m, :], ps_ep[:], AF.Identity,
                                             bias=sb_enc[:, m, :])
                        tm = f1c.tile([128, LC], bfl, tag="tm", name="tm")
                        nc.gpsimd.tensor_tensor(tm[:], dxs[:], sg[:], OP.mult)
                        nc.gpsimd.tensor_tensor(comb[:, m, :], comb[:, m, :], tm[:],
                                                OP.add)
                    else:
                        tm = f1c.tile([128, LC], bfl, tag="tm", name="tm")
                        nc.vector.scalar_tensor_tensor(tm[:], ps_dx[:],
                                                       sb_dec_x[:, m, :], sg[:],
                                                       OP.add, OP.mult)
                        nc.vector.scalar_tensor_tensor(comb[:, m, :], ps_ep[:],
                                                       sb_enc[:, m, :], tm[:],
                                                       OP.add, OP.add)
                return comb

            def front_rest_chunk(lc, comb, act_bias):
                # in_proj (own slice), conv (PE diag), padded x_proj partial,
                # stage partial x_dbl for the per-quarter AllReduce.
                ls = ts(lc, LC)
                half, i = lc // NLH, lc % NLH
                xm = fpool.tile([128, 3 + LC], bfl, tag="xm", name="xm")
                if lc == 0:
                    nc.gpsimd.memset(xm[:, 0:3], 0.0)
                else:
                    nc.scalar.activation(xm[:, 0:3],
                                         fstate["xm_prev"][:, LC:LC + 3],
                                         AF.Identity)
                ps_xm = psA.tile([128, LC], fp32, tag="mm", name="ps_xm")
                for t in range(4):
                    nc.tensor.matmul(ps_xm[:], sw_in_x[:, t, :],
                                     comb[:, t, :], start=(t == 0), stop=(t == 3))
                if act_bias:
                    nc.scalar.activation(xm[:, 3:3 + LC], ps_xm[:],
                                         AF.Identity, bias=sb_in_x[:, 0:1])
                else:
                    nc.vector.tensor_scalar(xm[:, 3:3 + LC], ps_xm[:],
                                            sb_in_x[:, 0:1], None, OP.add)
                ps_z = psA.tile([128, LC], fp32, tag="mm", name="ps_z")
                for t in range(4):
                    nc.tensor.matmul(ps_z[:], sw_in_z[:, t, :], comb[:, t, :],
                                     start=(t == 0), stop=(t == 3))
                nc.scalar.activation(s_siluz[:, ls], ps_z[:], AF.Silu,
                                     bias=sb_in_z[:, 0:1])
                # conv on PE: 4 shift-accumulated diagonal matmuls (own slice)
                ps_c = psA.tile([128, LC], fp32, tag="mm", name="ps_c")
                for k in range(KC):
                    nc.tensor.matmul(ps_c[:], sw_cd[:, k, :], xm[:, k:k + LC],
                                     start=(k == 0), stop=(k == KC - 1))
                nc.scalar.activation(s_uown[:, ls], ps_c[:], AF.Silu,
                                     bias=s_convb[:, 0:1])
                fstate["xm_prev"] = xm
                # padded x_proj partial: out rows [b*64, b*64+64) hold x_dbl
                ps_xd = psA.tile([128, LC], fp32, tag="mm", name="ps_xd")
                nc.tensor.matmul(ps_xd[:], sw_xp[:, :], s_uown[:, ls],
                                 start=True, stop=True)
                ar_st = f1c.tile([128, LC], bfl, tag="ar_st", name="ar_st")
                nc.scalar.activation(ar_st[:], ps_xd[:], AF.Copy)
                nc.sync.dma_start(out=ar_in_h[half][:, ts(i, LC)], in_=ar_st[:])
                if i == NLH - 1:
                    nc.gpsimd.collective_compute(
                        "AllReduce", OP.add, replica_groups=G8,
                        ins=[ar_in_h[half][:, :]], outs=[ar_out_h[half][:, :]],
                    )

            def softplus_half(half):
                # dt matmul over all 128 AllReduced rows (other batch rows are
                # zeroed in sw_dt), then softplus(x) = ln(1 + exp(x + dt_b))
                hs = ts(half, LH)
                dtin = f1c.tile([128, LH], bfl, tag="dtin", name="dtin")
                nc.sync.dma_start(out=dtin[:], in_=ar_out_h[half][:, :])
                fstate[f"xdbl{half}"] = dtin
                for fc in range(4):
                    ps_dt = psA.tile([128, LC], fp32, tag="mm", name="ps_dt")
                    nc.tensor.matmul(ps_dt[:], sw_dt[:, :], dtin[:, ts(fc, LC)],
                                     start=True, stop=True)
                    nc.scalar.activation(s_dtraw[:, ts(fc, LC)], ps_dt[:], AF.Copy)
                nc.scalar.activation(s_ex[:], s_dtraw[:], AF.Exp, bias=sb_dt[:, 0:1])
                nc.scalar.activation(s_dt[:, hs], s_ex[:], AF.Ln, bias=1.0)

            def front(half):
                for i in range(NLH):
                    lc = half * NLH + i
                    comb = ph1_chunk(lc)
                    front_rest_chunk(lc, comb, act_bias=False)
                if half == 0:
                    softplus_half(half)

            spool = stack.enter_context(tc.tile_pool(name="scan", bufs=1))
            sbc = stack.enter_context(tc.tile_pool(name="scanbc", bufs=2))
            sy = stack.enter_context(tc.tile_pool(name="scany", bufs=1))

            def scan(half, fillers=None):
                fillers = fillers or {}
                hs = ts(half, LH)
                s_dtu = sy.tile([128, LH], bfl, tag="dtu", name="s_dtu")
                nc.vector.tensor_tensor(s_dtu[:], s_dt[:, hs], s_uown[:, hs], OP.mult)
                s_uD = sy.tile([128, LH], bfl, tag="uD", name="s_uD")
                nc.vector.tensor_scalar(s_uD[:], s_uown[:, hs], s_d[:, 0:1], None,
                                        OP.mult)
                ysum = sy.tile([128, LH], bfl, tag="ysum", name="ysum")
                xdbl = fstate[f"xdbl{half}"]
                for n in range(N):
                    w = n % 2
                    bc2 = sbc.tile([128, 2, LH], bfl, tag="bc2", name="bc2")
                    for s in range(2):          # 0: B row, 1: C row
                        for fc in range(4):
                            ps_bc = psA.tile([128, LC], fp32, tag="mm",
                                             name="ps_bc")
                            nc.tensor.matmul(ps_bc[:], sw_sel[:, s * N + n, :],
                                             xdbl[:, ts(fc, LC)],
                                             start=True, stop=True)
                            nc.scalar.activation(bc2[:, s, ts(fc, LC)], ps_bc[:],
                                                 AF.Copy)
                    bbc = bc2[:, 0, :]
                    cbc = bc2[:, 1, :]
                    a = spool.tile([128, LH], bfl, tag=f"a{n % 3}", name="a")
                    nc.scalar.activation(a[:], s_dt[:, hs], AF.Exp,
                                         scale=s_a[:, n:n + 1])
                    bt = spool.tile([128, LH], bfl, tag=f"b{w}", name=f"b{w}")
                    nc.vector.tensor_tensor(bt[:], s_dtu[:], bbc, OP.mult)
                    h = spool.tile([128, LH], bfl, tag=f"h{w}", name=f"h{w}")
                    nc.vector.tensor_tensor_scan(h[:], a[:], bt[:],
                                                 carry[:, n:n + 1], OP.mult, OP.add)
                    if half == 0:
                        nc.vector.tensor_copy(carry[:, n:n + 1], h[:, LH - 1:LH])
                    nc.vector.tensor_tensor(h[:], h[:], cbc, OP.mult)
                    if n == 0:
                        nc.vector.tensor_tensor(ysum[:], h[:], s_uD[:], OP.add)
                    else:
                        nc.vector.tensor_tensor(ysum[:], ysum[:], h[:], OP.add)
                    if n in fillers:
                        fillers[n]()
                y_h = sy.tile([128, LH], bfl, tag="y", name="y_h")
                nc.vector.tensor_tensor(y_h[:], ysum[:], s_siluz[:, hs], OP.mult)
                for j in range(8):
                    nc.sync.dma_start(out=a2a_in[half][j * 128:(j + 1) * 128, :],
                                      in_=y_h[:, ts(j % 4, TQ)])
                nc.gpsimd.collective_compute(
                    "AllToAll", mybir.AluOpType.bypass, replica_groups=G8,
                    ins=[a2a_in[half][:, :]], outs=[a2a_out[half][:, :]],
                )

            tpool = stack.enter_context(tc.tile_pool(name="tail", bufs=1))

            tstate = {}

            def tail_main(half):
                hs = ts(half, TQ)
                s_yall = tpool.tile([128, 8, TQ], bfl, tag="yall", name="s_yall")
                a2a_r = a2a_out[half][:, :].rearrange("(t k) l -> k t l", k=128)
                nc.sync.dma_start(out=s_yall[:], in_=a2a_r)
                s_gated = tpool.tile([128, 4, TQ], bfl, tag="gated", name="s_gated")
                for m in range(4):
                    ps_mo = psA.tile([128, TQ], fp32, tag="mm", name="ps_mo")
                    for t in range(8):
                        nc.tensor.matmul(ps_mo[:], sw_mo[:, t, ts(m, 128)],
                                         s_yall[:, t, :], start=(t == 0), stop=(t == 7))
                    spr = f1c.tile([128, TQ], bfl, tag="spr", name="spr")
                    nc.scalar.activation(spr[:], ps_mo[:], AF.Identity,
                                         bias=sb_mo[:, m, :])
                    nc.gpsimd.tensor_tensor(s_gated[:, m, :], spr[:],
                                            s_sgate[:, m, hs], OP.mult)
                s_res = tpool.tile([128, 2, TQ], bfl, tag=f"res{half}", name="s_res")
                ps_sum = psts.tile([1, TQ], fp32, tag="ps_sum", name="ps_sum")
                ps_sq = psts.tile([1, TQ], fp32, tag="ps_sq", name="ps_sq")
                for m in range(2):
                    ps_o = psA.tile([128, TQ], fp32, tag="mm", name="ps_o")
                    for t in range(4):
                        nc.tensor.matmul(ps_o[:], sw_out[:, t, ts(m, 128)],
                                         s_gated[:, t, :], start=(t == 0), stop=(t == 3))
                    otmp = f1c.tile([128, TQ], bfl, tag="otmp", name="otmp")
                    nc.scalar.activation(otmp[:], ps_o[:], AF.Identity,
                                         bias=sb_out[:, m, :])
                    nc.gpsimd.tensor_tensor(s_res[:, m, :], otmp[:],
                                            s_decf[:, m, hs], OP.add)
                    res2 = f1c.tile([128, TQ], bfl, tag="res2", name="res2")
                    nc.scalar.activation(res2[:], s_res[:, m, :], AF.Square)
                    nc.tensor.matmul(ps_sum[:], ones_bf[:], s_res[:, m, :],
                                     start=(m == 0), stop=(m == 1))
                    nc.tensor.matmul(ps_sq[:], ones_bf[:], res2[:],
                                     start=(m == 0), stop=(m == 1))
                mu = tpool.tile([1, TQ], bfl, tag="mu", name="mu")
                nc.scalar.activation(mu[:], ps_sum[:], AF.Copy, scale=1.0 / C)
                musq = tpool.tile([1, TQ], bfl, tag="musq", name="musq")
                nc.scalar.activation(musq[:], mu[:], AF.Square)
                sq_sb = tpool.tile([1, TQ], bfl, tag="sq_sb", name="sq_sb")
                nc.scalar.activation(sq_sb[:], ps_sq[:], AF.Copy, scale=1.0 / C)
                var = tpool.tile([1, TQ], bfl, tag="var", name="var")
                nc.gpsimd.tensor_tensor(var[:], sq_sb[:], musq[:], OP.subtract)
                sd = tpool.tile([1, TQ], fp32, tag=f"sd{half}", name="sd")
                nc.scalar.activation(sd[:], var[:], AF.Sqrt, bias=eps[:, 0:1])
                nc.sync.dma_start(out=ln_mu[half][0:1, :], in_=mu[:])
                tstate[half] = (s_res, sd)

            def tail_fin(half):
                hs = ts(half, TQ)
                s_res, sd = tstate[half]
                rstd = tpool.tile([1, TQ], fp32, tag="rstd", name="rstd")
                nc.vector.reciprocal(rstd[:], sd[:])
                nc.sync.dma_start(out=ln_rs[half][0:1, :], in_=rstd[:])
                mu_bc = tpool.tile([128, TQ], bfl, tag="mu_bc", name="mu_bc")
                rs_bc = tpool.tile([128, TQ], fp32, tag="rs_bc", name="rs_bc")
                lnm = ln_mu[half][0:1, 0:1]
                lnr = ln_rs[half][0:1, 0:1]
                nc.sync.dma_start(out=mu_bc[:], in_=bass.AP(
                    tensor=lnm.tensor, offset=lnm.offset, ap=[[0, 128], [1, TQ]]))
                nc.sync.dma_start(out=rs_bc[:], in_=bass.AP(
                    tensor=lnr.tensor, offset=lnr.offset, ap=[[0, 128], [1, TQ]]))
                for m in range(2):
                    nc.gpsimd.tensor_tensor(s_res[:, m, :], s_res[:, m, :], mu_bc[:],
                                            OP.subtract)
                    nc.gpsimd.tensor_tensor(s_res[:, m, :], s_res[:, m, :], rs_bc[:],
                                            OP.mult)
                    t1 = tpool.tile([128, TQ], fp32, tag="t1", name="t1")
                    nc.scalar.activation(t1[:], s_res[:, m, :], AF.Identity,
                                         scale=s_g[:, m, :], bias=s_bln[:, m, :])
                    nc.sync.dma_start(
                        out=res_out.ap().rearrange("(t k) l -> k t l", k=128)[:, m, hs],
                        in_=t1[:])

            # ---- emission order ----
            front(0)
            front(1)
            scan(0, {10: lambda: softplus_half(1)})
            scan(1, {14: lambda: tail_main(0)})
            tail_fin(0)
            tail_main(1)
            tail_fin(1)

    nc.compile()
    return nc


def _in_maps(inp):
    A = -np.exp(inp["A_log"].astype(np.float32))
    dec_T = inp["decoder_feat"].reshape(B, C, L)
    enc_T = inp["encoder_feat"].reshape(B, C, L)
    dec_T_bf = dec_T.astype(bf16)
    enc_T_bf = enc_T.astype(bf16)

    def col(x):
        return np.ascontiguousarray(np.asarray(x, np.float32).reshape(-1, 1))

    common = {
        "w_dec_x": np.ascontiguousarray(inp["dec_w"][:, :Di].astype(bf16)),
        "w_dec_g": np.ascontiguousarray(inp["dec_w"][:, Di:].astype(bf16)),
        "b_dec_x": col(inp["dec_b"][:Di]),
        "b_dec_g": col(inp["dec_b"][Di:]),
        "w_enc": inp["enc_w"].astype(bf16),
        "b_enc": col(inp["enc_b"]),
        "b_mo": col(inp["m_out_b"]),
        "w_out": inp["out_w"].astype(bf16),
        "b_out": col(inp["out_b"]),
        "g_col": col(inp["ln_g"]),
        "bln_col": col(inp["ln_b"]),
    }

    in_maps = []
    for c in range(NCORES):
        b, q = c // 4, c % 4
        ds = slice(q * DQ, (q + 1) * DQ)
        m = dict(common)
        m["dec_bf"] = dec_T_bf[b]
        m["enc_bf"] = enc_T_bf[b]
        tok = np.r_[np.arange(q * TQ, (q + 1) * TQ),
                    np.arange(LH + q * TQ, LH + (q + 1) * TQ)]
        m["dec_f32q"] = np.ascontiguousarray(dec_T[b][:, tok].astype(np.float32))
        m["w_in_x"] = np.ascontiguousarray(inp["in_w"][:, :Di][:, ds].astype(bf16))
        m["b_in_x"] = col(inp["in_b"][:Di][ds])
        m["w_in_z"] = np.ascontiguousarray(
            inp["in_w"][:, Di + q * DQ:Di + (q + 1) * DQ].astype(bf16))
        m["b_in_z"] = col(inp["in_b"][Di + q * DQ:Di + (q + 1) * DQ])
        cw = inp["conv_w"][ds, 0, :].astype(np.float32)     # (DQ, KC) own slice
        wcd = np.zeros((DQ, KC, DQ), np.float32)
        idx = np.arange(DQ)
        for k in range(KC):
            wcd[idx, k, idx] = cw[idx, k]
        m["w_cd"] = wcd.astype(bf16)
        m["conv_b"] = col(inp["conv_b"][ds])
        # padded x_proj: own 128 rows -> 128 out cols, my batch block only
        wxp = np.zeros((DQ, 128), np.float32)
        wxp[:, b * 64:b * 64 + 2 * R] = inp["x_proj_w"][ds, :]
        m["w_xp"] = wxp.astype(bf16)
        # padded dt weights: contraction over all 128 AR rows, other rows 0
        wdt = np.zeros((128, DQ), np.float32)
        wdt[b * 64:b * 64 + R, :] = inp["dt_w"][:, ds]
        m["w_dt"] = wdt.astype(bf16)
        m["b_dt"] = col(inp["dt_b"][ds])
        m["bsel"] = np.array([[1 - b]], np.int32)
        sel = np.zeros((128, 2 * N, 128), np.float32)
        for n in range(N):
            sel[b * 64 + R + n, n, :] = 1.0          # B row
            sel[b * 64 + R + N + n, N + n, :] = 1.0  # C row
        m["sel_bc"] = sel.astype(bf16)
        wmo8 = np.zeros((2 * Di, Di), np.float32)
        for r in range(8):
            if r // 4 == b:
                rq = r % 4
                wmo8[r * DQ:(r + 1) * DQ] = inp["m_out_w"][rq * DQ:(rq + 1) * DQ]
        m["w_mo"] = wmo8.astype(bf16)
        m["a_sl"] = np.ascontiguousarray(A[ds])
        m["d_col"] = col(inp["D_param"][ds])
        in_maps.append(m)
    return in_maps


def kernel(**inputs):
    from concourse.bass_utils import run_bass_kernel_spmd

    inp = {k: np.asarray(v) for k, v in inputs.items()}
    if "nc" not in _cache:
        _cache["nc"] = _build()
    res = run_bass_kernel_spmd(_cache["nc"], _in_maps(inp), list(range(NCORES)))
    out = np.zeros((B, C, L), np.float32)
    for c in range(NCORES):
        b, q = c // 4, c % 4
        r = res.results[c]["res"]
        out[b][:, q * TQ:(q + 1) * TQ] = r[:, 0:TQ]
        out[b][:, LH + q * TQ:LH + (q + 1) * TQ] = r[:, TQ:2 * TQ]
    return out.reshape(B, C, Hh, Ww)


def run_traced(inp):
    from concourse.bass_utils import run_bass_kernel_spmd

    if "nc" not in _cache:
        _cache["nc"] = _build()
    return run_bass_kernel_spmd(_cache["nc"], _in_maps(inp), list(range(NCORES)),
                                trace=True)


# revision 24
# speedup vs baseline: 1.1171x; 1.0837x over previous
"""CrossMambaFusion Trainium2 kernel — 8-core SPMD via bass/Tile. v2.

Sharding (hardcoded for B=2, C=256, H=W=64, Di=512, N=16, R=32, K=4):
  core c -> batch b = c//4, d_inner slice q = c%4 (128 channels).
  Feature-major [features, tokens] on device; (B,C,H,W) -> (C, L=4096).

v2 (vs v1 baseline):
  - L processed in two halves: the DVE selective scan of half 0 overlaps the
    front (proj/conv/dt) of half 1 on PE/ACT/Pool, and the AllToAll + tail of
    half 0 overlap the scan of half 1.
  - causal depthwise conv on PE as 4 shift-accumulated diagonal matmuls
    (removes ~86us of DVE stt chains).
  - softplus as 2 ACT ops: ln(1 + exp(x)) (x is small here; no overflow).
  - front/tail elementwise moved to Pool (gpsimd) so DVE does only ph1 stt,
    dtu and the scan, and the tail can run while DVE scans half 1.
  - per-half AllToAll [1024, 512]; token ownership interleaved so each core's
    tail input arrives right after its half's collective.

Token ownership: core (b, q) owns tokens {h*2048 + q*512 + [0,512) : h in 0,1}.
AllToAll in-block j = own y chunk of tokens [(j%4)*512, ...) of that half;
rows duplicated across batch groups; m_out weight rows of the other batch
group are zeroed (as in v1).
"""
import numpy as np
import ml_dtypes

bf16 = ml_dtypes.bfloat16

B, C, Hh, Ww = 2, 256, 64, 64
L = Hh * Ww
Di, N, R, KC = 512, 16, 32, 4
DQ = 128
NCORES = 8
LH = L // 2          # 2048
LC = 512             # front chunk
NLH = LH // LC       # 4 chunks per half
TQ = 512             # tokens per (core, half) piece

_cache = {}


def _build():
    import concourse.bass as bass
    import concourse.mybir as mybir
    import concourse.tile as tile
    from concourse import bacc

    fp32 = mybir.dt.float32
    bfl = mybir.dt.bfloat16
    AF = mybir.ActivationFunctionType
    OP = mybir.AluOpType
    ts = bass.ts

    nc = bacc.Bacc("TRN2", target_bir_lowering=False, num_devices=NCORES)

    def din(name, shape, dt=fp32):
        return nc.declare_dram_parameter(name, list(shape), dt, isOutput=False)

    dec_bf = din("dec_bf", (C, L), bfl)
    enc_bf = din("enc_bf", (C, L), bfl)
    dec_f32q = din("dec_f32q", (C, 2 * TQ), fp32)
    w_dec_x = din("w_dec_x", (C, Di), bfl)
    w_dec_g = din("w_dec_g", (C, Di), bfl)
    b_dec_x = din("b_dec_x", (Di, 1))
    b_dec_g = din("b_dec_g", (Di, 1))
    w_enc = din("w_enc", (C, Di), bfl)
    b_enc = din("b_enc", (Di, 1))
    w_in_x = din("w_in_x", (Di, DQ), bfl)      # own-slice columns
    b_in_x = din("b_in_x", (DQ, 1))            # own slice
    w_in_z = din("w_in_z", (Di, DQ), bfl)
    b_in_z = din("b_in_z", (DQ, 1))
    w_cd = din("w_cd", (DQ, KC, DQ), bfl)      # diagonal conv weights per k
    conv_b = din("conv_b", (DQ, 1))            # own slice
    w_xp = din("w_xp", (DQ, 128), bfl)         # own rows, cols zero-padded per batch
    w_dt = din("w_dt", (128, DQ), bfl)  # rows zero-padded per batch
    b_dt = din("b_dt", (DQ, 1))
    a_sl = din("a_sl", (DQ, N))
    sel_bc = din("sel_bc", (128, 2 * N, 128), bfl)  # one-hot B/C row selectors
    d_col = din("d_col", (DQ, 1))
    w_mo = din("w_mo", (2 * Di, Di), bfl)      # rows of other batch group zeroed
    b_mo = din("b_mo", (Di, 1))
    w_out = din("w_out", (Di, C), bfl)
    b_out = din("b_out", (C, 1))
    g_col = din("g_col", (C, 1))
    bln_col = din("bln_col", (C, 1))

    res_out = nc.declare_dram_parameter("res", [C, 2 * TQ], fp32, isOutput=True)

    with tile.TileContext(nc) as tc:
        import contextlib
        with contextlib.ExitStack() as stack:
            wpool = stack.enter_context(tc.tile_pool(name="weights", bufs=1))
            cpool = stack.enter_context(tc.tile_pool(name="consts", bufs=1))
            dpool = stack.enter_context(tc.tile_pool(name="drambuf", bufs=1, space="DRAM"))

            ar_in_h = [dpool.tile([128, LH], bfl, name=f"arin{j}") for j in range(2)]
            ar_out_h = [dpool.tile([128, LH], bfl, name=f"arout{j}") for j in range(2)]
            ln_mu = [dpool.tile([1, TQ], bfl, name=f"lnmu{h}") for h in range(2)]
            ln_rs = [dpool.tile([1, TQ], fp32, name=f"lnrs{h}") for h in range(2)]
            a2a_in = [dpool.tile([2 * Di, TQ], bfl, name=f"a2ai{h}") for h in range(2)]
            a2a_out = [dpool.tile([2 * Di, TQ], bfl, name=f"a2ao{h}") for h in range(2)]

            def wload(ap, kt, m, name):
                t = wpool.tile([128, kt, m], bfl, tag=name, name=name)
                nc.sync.dma_start(out=t[:], in_=ap.ap().rearrange("(t k) m -> k t m", k=128))
                return t

            sw_dec_x = wload(w_dec_x, 2, Di, "w_dec_x")
            sw_enc = wload(w_enc, 2, Di, "w_enc")
            sw_dec_g = wload(w_dec_g, 2, Di, "w_dec_g")
            sw_in_x = wload(w_in_x, 4, DQ, "w_in_x")
            sw_in_z = wload(w_in_z, 4, DQ, "w_in_z")
            sw_xp = wpool.tile([128, 128], bfl)
            nc.sync.dma_start(out=sw_xp[:], in_=w_xp.ap())
            sw_mo = wload(w_mo, 8, Di, "w_mo")
            sw_out = wload(w_out, 4, C, "w_out")
            sw_dt = wpool.tile([128, DQ], bfl)
            nc.sync.dma_start(out=sw_dt[:], in_=w_dt.ap())
            sw_sel = wpool.tile([128, 2 * N, 128], bfl)
            nc.sync.dma_start(out=sw_sel[:], in_=sel_bc.ap())
            sw_cd = wpool.tile([128, KC, DQ], bfl)
            nc.sync.dma_start(out=sw_cd[:], in_=w_cd.ap())

            def cload(ap, nt, name, cols=1):
                if nt == 1:
                    t = cpool.tile([128, cols], fp32, tag=name, name=name)
                    nc.sync.dma_start(out=t[:], in_=ap.ap())
                else:
                    t = cpool.tile([128, nt, cols], fp32, tag=name, name=name)
                    nc.sync.dma_start(out=t[:], in_=ap.ap().rearrange("(t k) o -> k t o", k=128))
                return t

            sb_dec_x = cload(b_dec_x, 4, "b_dec_x")
            sb_dec_g = cload(b_dec_g, 4, "b_dec_g")
            sb_enc = cload(b_enc, 4, "b_enc")
            sb_in_x = cload(b_in_x, 1, "b_in_x")
            sb_in_z = cload(b_in_z, 1, "b_in_z")
            s_convb = cload(conv_b, 1, "conv_b")
            sb_dt = cload(b_dt, 1, "b_dt")
            s_a = cload(a_sl, 1, "a_sl", cols=N)
            s_d = cload(d_col, 1, "d_col")
            sb_mo = cload(b_mo, 4, "b_mo")
            sb_out = cload(b_out, 2, "b_out")
            s_g = cload(g_col, 2, "g_col")
            s_bln = cload(bln_col, 2, "bln_col")

            # batch-select scalar -> sync-engine register for cond DMAs
            bsel = nc.declare_dram_parameter("bsel", [1, 1], mybir.dt.int32,
                                             isOutput=False)
            # persistent tiles
            ppool = stack.enter_context(tc.tile_pool(name="persist", bufs=1))
            s_dt = ppool.tile([128, L], bfl)
            s_siluz = ppool.tile([128, L], bfl)
            s_uown = ppool.tile([128, L], bfl)
            s_sgate = ppool.tile([128, 4, 2 * TQ], bfl)
            s_decf = ppool.tile([128, 2, 2 * TQ], fp32)
            carry = ppool.tile([128, N], fp32)
            nc.vector.memset(carry[:], 0.0)
            ones = ppool.tile([128, 1], fp32)
            nc.vector.memset(ones[:], 1.0)
            ones_bf = ppool.tile([128, 1], bfl)
            nc.vector.memset(ones_bf[:], 1.0)
            eps = ppool.tile([1, 1], fp32)
            nc.vector.memset(eps[:], 1e-5)
            s_bsel = ppool.tile([1, 1], mybir.dt.int32)
            nc.sync.dma_start(out=s_bsel[:], in_=bsel.ap())
            breg = nc.sync.alloc_register("bsel_reg")
            nc.sync.reg_load(breg, s_bsel[0:1, 0:1])
            bsnap = nc.sync.snap(breg, min_val=0, max_val=1)
            zreg = nc.sync.alloc_register("zero_reg")
            nc.sync.reg_mov(zreg, 0)
            zsnap = nc.sync.snap(zreg, min_val=0, max_val=0)
            cond_b0 = bsnap != zsnap   # true on batch-0 cores (bsel=1)
            cond_b1 = bsnap == zsnap

            # rotating pools (shared across phases)
            fpool = stack.enter_context(tc.tile_pool(name="front", bufs=2))
            f1c = stack.enter_context(tc.tile_pool(name="small", bufs=2))
            psA = stack.enter_context(tc.tile_pool(name="psA", bufs=6, space="PSUM"))
            psts = stack.enter_context(tc.tile_pool(name="psts", bufs=1, space="PSUM"))

            # ---- pretail: decoder gate on own tokens (sigmoid table) ----
            nc.sync.dma_start(out=s_decf[:],
                              in_=dec_f32q.ap().rearrange("(t k) l -> k t l", k=128))
            with tc.tile_pool(name="pret", bufs=1) as prepool:
                s_decq = prepool.tile([128, 2, 2 * TQ], bfl)
                nc.gpsimd.tensor_copy(s_decq[:], s_decf[:])
                for lc in range(2):
                    ls = ts(lc, TQ)
                    for m in range(4):
                        ps_g = psA.tile([128, TQ], fp32, tag="mm", name="ps_g")
                        for t in range(2):
                            nc.tensor.matmul(ps_g[:], sw_dec_g[:, t, ts(m, 128)],
                                             s_decq[:, t, ls], start=(t == 0), stop=(t == 1))
                        nc.scalar.activation(s_sgate[:, m, ls], ps_g[:], AF.Sigmoid,
                                             bias=sb_dec_g[:, m, :])

            dec_r = dec_bf.ap().rearrange("(t k) l -> k t l", k=128)
            enc_r = enc_bf.ap().rearrange("(t k) l -> k t l", k=128)
            fstate = {}
            G8 = [[0, 1, 2, 3, 4, 5, 6, 7]]
            BROW = 32   # x_dbl rows: [0:32)=dt_in, [32:48)=B, [48:64)=C
            s_dtraw = ppool.tile([128, LH], bfl)   # staged softplus input (1 half)
            s_ex = ppool.tile([128, LH], bfl)      # softplus exp scratch

            def ph1_chunk(lc, on_pool=False):
                # combined = dec_x*sig(enc_p) + enc_p for one chunk
                ls = ts(lc, LC)
                s_dec = fpool.tile([128, 2, LC], bfl, tag="s_dec", name="s_dec")
                s_enc = fpool.tile([128, 2, LC], bfl, tag="s_enc", name="s_enc")
                nc.sync.dma_start(out=s_dec[:], in_=dec_r[:, :, ls])
                nc.sync.dma_start(out=s_enc[:], in_=enc_r[:, :, ls])
                comb = fpool.tile([128, 4, LC], bfl, tag="comb", name="comb")
                for m in range(4):
                    ps_dx = psA.tile([128, LC], fp32, tag="mm", name="ps_dx")
                    ps_ep = psA.tile([128, LC], fp32, tag="mm", name="ps_ep")
                    for t in range(2):
                        nc.tensor.matmul(ps_dx[:], sw_dec_x[:, t, ts(m, 128)],
                                         s_dec[:, t, :], start=(t == 0), stop=(t == 1))
                    for t in range(2):
                        nc.tensor.matmul(ps_ep[:], sw_enc[:, t, ts(m, 128)],
                                         s_enc[:, t, :], start=(t == 0), stop=(t == 1))
                    sg = f1c.tile([128, LC], bfl, tag="sg", name="sg")
                    nc.scalar.activation(sg[:], ps_ep[:], AF.Sigmoid,
                                         bias=sb_enc[:, m, :])
                    if on_pool:
                        # keep DVE free during the scan: materialize biased
                        # dx/ep via ACT (table-free), combine on Pool
                        dxs = f1c.tile([128, LC], bfl, tag="dxs", name="dxs")
                        nc.scalar.activation(dxs[:], ps_dx[:], AF.Identity,
                                             bias=sb_dec_x[:, m, :])
                        nc.scalar.activation(comb[:, m, :], ps_ep[:], AF.Identity,
                                             bias=sb_enc[:, m, :])
                        tm = f1c.tile([128, LC], bfl, tag="tm", name="tm")
                        nc.gpsimd.tensor_tensor(tm[:], dxs[:], sg[:], OP.mult)
                        nc.gpsimd.tensor_tensor(comb[:, m, :], comb[:, m, :], tm[:],
                                                OP.add)
                    else:
                        tm = f1c.tile([128, LC], bfl, tag="tm", name="tm")
                        nc.vector.scalar_tensor_tensor(tm[:], ps_dx[:],
                                                       sb_dec_x[:, m, :], sg[:],
                                                       OP.add, OP.mult)
                        nc.vector.scalar_tensor_tensor(comb[:, m, :], ps_ep[:],
                                                       sb_enc[:, m, :], tm[:],
                                                       OP.add, OP.add)
                return comb

            def front_rest_chunk(lc, comb, act_bias):
                # in_proj (own slice), conv (PE diag), padded x_proj partial,
                # stage partial x_dbl for the per-quarter AllReduce.
                ls = ts(lc, LC)
                half, i = lc // NLH, lc % NLH
                xm = fpool.tile([128, 3 + LC], bfl, tag="xm", name="xm")
                if lc == 0:
                    nc.gpsimd.memset(xm[:, 0:3], 0.0)
                else:
                    nc.scalar.activation(xm[:, 0:3],
                                         fstate["xm_prev"][:, LC:LC + 3],
                                         AF.Identity)
                ps_xm = psA.tile([128, LC], fp32, tag="mm", name="ps_xm")
                for t in range(4):
                    nc.tensor.matmul(ps_xm[:], sw_in_x[:, t, :],
                                     comb[:, t, :], start=(t == 0), stop=(t == 3))
                if act_bias:
                    nc.scalar.activation(xm[:, 3:3 + LC], ps_xm[:],
                                         AF.Identity, bias=sb_in_x[:, 0:1])
                else:
                    nc.vector.tensor_scalar(xm[:, 3:3 + LC], ps_xm[:],
                                            sb_in_x[:, 0:1], None, OP.add)
                ps_z = psA.tile([128, LC], fp32, tag="mm", name="ps_z")
                for t in range(4):
                    nc.tensor.matmul(ps_z[:], sw_in_z[:, t, :], comb[:, t, :],
                                     start=(t == 0), stop=(t == 3))
                nc.scalar.activation(s_siluz[:, ls], ps_z[:], AF.Silu,
                                     bias=sb_in_z[:, 0:1])
                # conv on PE: 4 shift-accumulated diagonal matmuls (own slice)
                ps_c = psA.tile([128, LC], fp32, tag="mm", name="ps_c")
                for k in range(KC):
                    nc.tensor.matmul(ps_c[:], sw_cd[:, k, :], xm[:, k:k + LC],
                                     start=(k == 0), stop=(k == KC - 1))
                nc.scalar.activation(s_uown[:, ls], ps_c[:], AF.Silu,
                                     bias=s_convb[:, 0:1])
                fstate["xm_prev"] = xm
                # padded x_proj partial: out rows [b*64, b*64+64) hold x_dbl
                ps_xd = psA.tile([128, LC], fp32, tag="mm", name="ps_xd")
                nc.tensor.matmul(ps_xd[:], sw_xp[:, :], s_uown[:, ls],
                                 start=True, stop=True)
                ar_st = f1c.tile([128, LC], bfl, tag="ar_st", name="ar_st")
                nc.scalar.activation(ar_st[:], ps_xd[:], AF.Copy)
                nc.sync.dma_start(out=ar_in_h[half][:, ts(i, LC)], in_=ar_st[:])
                if i == NLH - 1:
                    nc.gpsimd.collective_compute(
                        "AllReduce", OP.add, replica_groups=G8,
                        ins=[ar_in_h[half][:, :]], outs=[ar_out_h[half][:, :]],
                    )

            def softplus_half(half):
                # dt matmul over all 128 AllReduced rows (other batch rows are
                # zeroed in sw_dt), then softplus(x) = ln(1 + exp(x + dt_b))
                hs = ts(half, LH)
                dtin = f1c.tile([128, LH], bfl, tag="dtin", name="dtin")
                nc.sync.dma_start(out=dtin[:], in_=ar_out_h[half][:, :])
                fstate[f"xdbl{half}"] = dtin
                for fc in range(4):
                    ps_dt = psA.tile([128, LC], fp32, tag="mm", name="ps_dt")
                    nc.tensor.matmul(ps_dt[:], sw_dt[:, :], dtin[:, ts(fc, LC)],
                                     start=True, stop=True)
                    nc.scalar.activation(s_dtraw[:, ts(fc, LC)], ps_dt[:], AF.Copy)
                nc.scalar.activation(s_ex[:], s_dtraw[:], AF.Exp, bias=sb_dt[:, 0:1])
                nc.scalar.activation(s_dt[:, hs], s_ex[:], AF.Ln, bias=1.0)

            def front(half):
                for i in range(NLH):
                    lc = half * NLH + i
                    comb = ph1_chunk(lc)
                    front_rest_chunk(lc, comb, act_bias=False)

            spool = stack.enter_context(tc.tile_pool(name="scan", bufs=1))
            sbc = stack.enter_context(tc.tile_pool(name="scanbc", bufs=2))
            sy = stack.enter_context(tc.tile_pool(name="scany", bufs=1))

            def scan(half, fillers=None):
                fillers = fillers or {}
                hs = ts(half, LH)
                s_dtu = sy.tile([128, LH], bfl, tag="dtu", name="s_dtu")
                nc.vector.tensor_tensor(s_dtu[:], s_dt[:, hs], s_uown[:, hs], OP.mult)
                s_uD = sy.tile([128, LH], bfl, tag="uD", name="s_uD")
                nc.vector.tensor_scalar(s_uD[:], s_uown[:, hs], s_d[:, 0:1], None,
                                        OP.mult)
                ysum = sy.tile([128, LH], bfl, tag="ysum", name="ysum")
                xdbl = fstate[f"xdbl{half}"]
                for n in range(N):
                    w = n % 2
                    bc2 = sbc.tile([128, 2, LH], bfl, tag="bc2", name="bc2")
                    for s in range(2):          # 0: B row, 1: C row
                        for fc in range(4):
                            ps_bc = psA.tile([128, LC], fp32, tag="mm",
                                             name="ps_bc")
                            nc.tensor.matmul(ps_bc[:], sw_sel[:, s * N + n, :],
                                             xdbl[:, ts(fc, LC)],
                                             start=True, stop=True)
                            nc.scalar.activation(bc2[:, s, ts(fc, LC)], ps_bc[:],
                                                 AF.Copy)
                    bbc = bc2[:, 0, :]
                    cbc = bc2[:, 1, :]
                    a = spool.tile([128, LH], bfl, tag=f"a{n % 3}", name="a")
                    nc.scalar.activation(a[:], s_dt[:, hs], AF.Exp,
                                         scale=s_a[:, n:n + 1])
                    bt = spool.tile([128, LH], bfl, tag=f"b{w}", name=f"b{w}")
                    nc.vector.tensor_tensor(bt[:], s_dtu[:], bbc, OP.mult)
                    h = spool.tile([128, LH], bfl, tag=f"h{w}", name=f"h{w}")
                    nc.vector.tensor_tensor_scan(h[:], a[:], bt[:],
                                                 carry[:, n:n + 1], OP.mult, OP.add)
                    if half == 0:
                        nc.vector.tensor_copy(carry[:, n:n + 1], h[:, LH - 1:LH])
                    nc.vector.tensor_tensor(h[:], h[:], cbc, OP.mult)
                    if n == 0:
                        nc.vector.tensor_tensor(ysum[:], h[:], s_uD[:], OP.add)
                    else:
                        nc.vector.tensor_tensor(ysum[:], ysum[:], h[:], OP.add)
                    if n in fillers:
                        fillers[n]()
                y_h = sy.tile([128, LH], bfl, tag="y", name="y_h")
                nc.vector.tensor_tensor(y_h[:], ysum[:], s_siluz[:, hs], OP.mult)
                for j in range(8):
                    nc.sync.dma_start(out=a2a_in[half][j * 128:(j + 1) * 128, :],
                                      in_=y_h[:, ts(j % 4, TQ)])
                nc.gpsimd.collective_compute(
                    "AllToAll", mybir.AluOpType.bypass, replica_groups=G8,
                    ins=[a2a_in[half][:, :]], outs=[a2a_out[half][:, :]],
                )

            tpool = stack.enter_context(tc.tile_pool(name="tail", bufs=1))

            tstate = {}

            def tail_main(half):
                hs = ts(half, TQ)
                s_yall = tpool.tile([128, 8, TQ], bfl, tag="yall", name="s_yall")
                a2a_r = a2a_out[half][:, :].rearrange("(t k) l -> k t l", k=128)
                nc.sync.dma_start(out=s_yall[:], in_=a2a_r)
                s_gated = tpool.tile([128, 4, TQ], bfl, tag="gated", name="s_gated")
                for m in range(4):
                    ps_mo = psA.tile([128, TQ], fp32, tag="mm", name="ps_mo")
                    for t in range(8):
                        nc.tensor.matmul(ps_mo[:], sw_mo[:, t, ts(m, 128)],
                                         s_yall[:, t, :], start=(t == 0), stop=(t == 7))
                    spr = f1c.tile([128, TQ], bfl, tag="spr", name="spr")
                    nc.scalar.activation(spr[:], ps_mo[:], AF.Identity,
                                         bias=sb_mo[:, m, :])
                    nc.gpsimd.tensor_tensor(s_gated[:, m, :], spr[:],
                                            s_sgate[:, m, hs], OP.mult)
                s_res = tpool.tile([128, 2, TQ], bfl, tag=f"res{half}", name="s_res")
                ps_sum = psts.tile([1, TQ], fp32, tag="ps_sum", name="ps_sum")
                ps_sq = psts.tile([1, TQ], fp32, tag="ps_sq", name="ps_sq")
                for m in range(2):
                    ps_o = psA.tile([128, TQ], fp32, tag="mm", name="ps_o")
                    for t in range(4):
                        nc.tensor.matmul(ps_o[:], sw_out[:, t, ts(m, 128)],
                                         s_gated[:, t, :], start=(t == 0), stop=(t == 3))
                    otmp = f1c.tile([128, TQ], bfl, tag="otmp", name="otmp")
                    nc.scalar.activation(otmp[:], ps_o[:], AF.Identity,
                                         bias=sb_out[:, m, :])
                    nc.gpsimd.tensor_tensor(s_res[:, m, :], otmp[:],
                                            s_decf[:, m, hs], OP.add)
                    res2 = f1c.tile([128, TQ], bfl, tag="res2", name="res2")
                    nc.scalar.activation(res2[:], s_res[:, m, :], AF.Square)
                    nc.tensor.matmul(ps_sum[:], ones_bf[:], s_res[:, m, :],
                                     start=(m == 0), stop=(m == 1))
                    nc.tensor.matmul(ps_sq[:], ones_bf[:], res2[:],
                                     start=(m == 0), stop=(m == 1))
                mu = tpool.tile([1, TQ], bfl, tag="mu", name="mu")
                nc.scalar.activation(mu[:], ps_sum[:], AF.Copy, scale=1.0 / C)
                musq = tpool.tile([1, TQ], bfl, tag="musq", name="musq")
                nc.scalar.activation(musq[:], mu[:], AF.Square)
                sq_sb = tpool.tile([1, TQ], bfl, tag="sq_sb", name="sq_sb")
                nc.scalar.activation(sq_sb[:], ps_sq[:], AF.Copy, scale=1.0 / C)
                var = tpool.tile([1, TQ], bfl, tag="var", name="var")
                nc.gpsimd.tensor_tensor(var[:], sq_sb[:], musq[:], OP.subtract)
                sd = tpool.tile([1, TQ], fp32, tag=f"sd{half}", name="sd")
                nc.scalar.activation(sd[:], var[:], AF.Sqrt, bias=eps[:, 0:1])
                nc.sync.dma_start(out=ln_mu[half][0:1, :], in_=mu[:])
                tstate[half] = (s_res, sd)

            def tail_fin(half):
                hs = ts(half, TQ)
                s_res, sd = tstate[half]
                rstd = tpool.tile([1, TQ], fp32, tag="rstd", name="rstd")
                nc.vector.reciprocal(rstd[:], sd[:])
                nc.sync.dma_start(out=ln_rs[half][0:1, :], in_=rstd[:])
                mu_bc = tpool.tile([128, TQ], bfl, tag="mu_bc", name="mu_bc")
                rs_bc = tpool.tile([128, TQ], fp32, tag="rs_bc", name="rs_bc")
                lnm = ln_mu[half][0:1, 0:1]
                lnr = ln_rs[half][0:1, 0:1]
                nc.sync.dma_start(out=mu_bc[:], in_=bass.AP(
                    tensor=lnm.tensor, offset=lnm.offset, ap=[[0, 128], [1, TQ]]))
                nc.sync.dma_start(out=rs_bc[:], in_=bass.AP(
                    tensor=lnr.tensor, offset=lnr.offset, ap=[[0, 128], [1, TQ]]))
                for m in range(2):
                    nc.gpsimd.tensor_tensor(s_res[:, m, :], s_res[:, m, :], mu_bc[:],
                                            OP.subtract)
                    nc.gpsimd.tensor_tensor(s_res[:, m, :], s_res[:, m, :], rs_bc[:],
                                            OP.mult)
                    t1 = tpool.tile([128, TQ], fp32, tag="t1", name="t1")
                    nc.scalar.activation(t1[:], s_res[:, m, :], AF.Identity,
                                         scale=s_g[:, m, :], bias=s_bln[:, m, :])
                    nc.sync.dma_start(
                        out=res_out.ap().rearrange("(t k) l -> k t l", k=128)[:, m, hs],
                        in_=t1[:])

            # ---- emission order ----
            front(0)
            front(1)
            softplus_half(0)
            scan(0, {10: lambda: softplus_half(1)})
            scan(1, {14: lambda: tail_main(0)})
            tail_fin(0)
            tail_main(1)
            tail_fin(1)

    nc.compile()
    return nc


def _in_maps(inp):
    A = -np.exp(inp["A_log"].astype(np.float32))
    dec_T = inp["decoder_feat"].reshape(B, C, L)
    enc_T = inp["encoder_feat"].reshape(B, C, L)
    dec_T_bf = dec_T.astype(bf16)
    enc_T_bf = enc_T.astype(bf16)

    def col(x):
        return np.ascontiguousarray(np.asarray(x, np.float32).reshape(-1, 1))

    common = {
        "w_dec_x": np.ascontiguousarray(inp["dec_w"][:, :Di].astype(bf16)),
        "w_dec_g": np.ascontiguousarray(inp["dec_w"][:, Di:].astype(bf16)),
        "b_dec_x": col(inp["dec_b"][:Di]),
        "b_dec_g": col(inp["dec_b"][Di:]),
        "w_enc": inp["enc_w"].astype(bf16),
        "b_enc": col(inp["enc_b"]),
        "b_mo": col(inp["m_out_b"]),
        "w_out": inp["out_w"].astype(bf16),
        "b_out": col(inp["out_b"]),
        "g_col": col(inp["ln_g"]),
        "bln_col": col(inp["ln_b"]),
    }

    in_maps = []
    for c in range(NCORES):
        b, q = c // 4, c % 4
        ds = slice(q * DQ, (q + 1) * DQ)
        m = dict(common)
        m["dec_bf"] = dec_T_bf[b]
        m["enc_bf"] = enc_T_bf[b]
        tok = np.r_[np.arange(q * TQ, (q + 1) * TQ),
                    np.arange(LH + q * TQ, LH + (q + 1) * TQ)]
        m["dec_f32q"] = np.ascontiguousarray(dec_T[b][:, tok].astype(np.float32))
        m["w_in_x"] = np.ascontiguousarray(inp["in_w"][:, :Di][:, ds].astype(bf16))
        m["b_in_x"] = col(inp["in_b"][:Di][ds])
        m["w_in_z"] = np.ascontiguousarray(
            inp["in_w"][:, Di + q * DQ:Di + (q + 1) * DQ].astype(bf16))
        m["b_in_z"] = col(inp["in_b"][Di + q * DQ:Di + (q + 1) * DQ])
        cw = inp["conv_w"][ds, 0, :].astype(np.float32)     # (DQ, KC) own slice
        wcd = np.zeros((DQ, KC, DQ), np.float32)
        idx = np.arange(DQ)
        for k in range(KC):
            wcd[idx, k, idx] = cw[idx, k]
        m["w_cd"] = wcd.astype(bf16)
        m["conv_b"] = col(inp["conv_b"][ds])
        # padded x_proj: own 128 rows -> 128 out cols, my batch block only
        wxp = np.zeros((DQ, 128), np.float32)
        wxp[:, b * 64:b * 64 + 2 * R] = inp["x_proj_w"][ds, :]
        m["w_xp"] = wxp.astype(bf16)
        # padded dt weights: contraction over all 128 AR rows, other rows 0
        wdt = np.zeros((128, DQ), np.float32)
        wdt[b * 64:b * 64 + R, :] = inp["dt_w"][:, ds]
        m["w_dt"] = wdt.astype(bf16)
        m["b_dt"] = col(inp["dt_b"][ds])
        m["bsel"] = np.array([[1 - b]], np.int32)
        sel = np.zeros((128, 2 * N, 128), np.float32)
        for n in range(N):
            sel[b * 64 + R + n, n, :] = 1.0          # B row
            sel[b * 64 + R + N + n, N + n, :] = 1.0  # C row
        m["sel_bc"] = sel.astype(bf16)
        wmo8 = np.zeros((2 * Di, Di), np.float32)
        for r in range(8):
            if r // 4 == b:
                rq = r % 4
                wmo8[r * DQ:(r + 1) * DQ] = inp["m_out_w"][rq * DQ:(rq + 1) * DQ]
        m["w_mo"] = wmo8.astype(bf16)
        m["a_sl"] = np.ascontiguousarray(A[ds])
        m["d_col"] = col(inp["D_param"][ds])
        in_maps.append(m)
    return in_maps


def kernel(**inputs):
    from concourse.bass_utils import run_bass_kernel_spmd

    inp = {k: np.asarray(v) for k, v in inputs.items()}
    if "nc" not in _cache:
        _cache["nc"] = _build()
    res = run_bass_kernel_spmd(_cache["nc"], _in_maps(inp), list(range(NCORES)))
    out = np.zeros((B, C, L), np.float32)
    for c in range(NCORES):
        b, q = c // 4, c % 4
        r = res.results[c]["res"]
        out[b][:, q * TQ:(q + 1) * TQ] = r[:, 0:TQ]
        out[b][:, LH + q * TQ:LH + (q + 1) * TQ] = r[:, TQ:2 * TQ]
    return out.reshape(B, C, Hh, Ww)


def run_traced(inp):
    from concourse.bass_utils import run_bass_kernel_spmd

    if "nc" not in _cache:
        _cache["nc"] = _build()
    return run_bass_kernel_spmd(_cache["nc"], _in_maps(inp), list(range(NCORES)),
                                trace=True)


# revision 25
# speedup vs baseline: 1.1450x; 1.0249x over previous
"""CrossMambaFusion Trainium2 kernel — 8-core SPMD via bass/Tile. v2.

Sharding (hardcoded for B=2, C=256, H=W=64, Di=512, N=16, R=32, K=4):
  core c -> batch b = c//4, d_inner slice q = c%4 (128 channels).
  Feature-major [features, tokens] on device; (B,C,H,W) -> (C, L=4096).

v2 (vs v1 baseline):
  - L processed in two halves: the DVE selective scan of half 0 overlaps the
    front (proj/conv/dt) of half 1 on PE/ACT/Pool, and the AllToAll + tail of
    half 0 overlap the scan of half 1.
  - causal depthwise conv on PE as 4 shift-accumulated diagonal matmuls
    (removes ~86us of DVE stt chains).
  - softplus as 2 ACT ops: ln(1 + exp(x)) (x is small here; no overflow).
  - front/tail elementwise moved to Pool (gpsimd) so DVE does only ph1 stt,
    dtu and the scan, and the tail can run while DVE scans half 1.
  - per-half AllToAll [1024, 512]; token ownership interleaved so each core's
    tail input arrives right after its half's collective.

Token ownership: core (b, q) owns tokens {h*2048 + q*512 + [0,512) : h in 0,1}.
AllToAll in-block j = own y chunk of tokens [(j%4)*512, ...) of that half;
rows duplicated across batch groups; m_out weight rows of the other batch
group are zeroed (as in v1).
"""
import numpy as np
import ml_dtypes

bf16 = ml_dtypes.bfloat16

B, C, Hh, Ww = 2, 256, 64, 64
L = Hh * Ww
Di, N, R, KC = 512, 16, 32, 4
DQ = 128
NCORES = 8
LH = L // 2          # 2048
LC = 512             # front chunk
NLH = LH // LC       # 4 chunks per half
TQ = 512             # tokens per (core, half) piece

_cache = {}


def _build():
    import concourse.bass as bass
    import concourse.mybir as mybir
    import concourse.tile as tile
    from concourse import bacc

    fp32 = mybir.dt.float32
    bfl = mybir.dt.bfloat16
    AF = mybir.ActivationFunctionType
    OP = mybir.AluOpType
    ts = bass.ts

    nc = bacc.Bacc("TRN2", target_bir_lowering=False, num_devices=NCORES)

    def din(name, shape, dt=fp32):
        return nc.declare_dram_parameter(name, list(shape), dt, isOutput=False)

    dec_bf = din("dec_bf", (C, L), bfl)
    enc_bf = din("enc_bf", (C, L), bfl)
    dec_f32q = din("dec_f32q", (C, 2 * TQ), fp32)
    w_dec_x = din("w_dec_x", (C, Di), bfl)
    w_dec_g = din("w_dec_g", (C, Di), bfl)
    b_dec_x = din("b_dec_x", (Di, 1))
    b_dec_g = din("b_dec_g", (Di, 1))
    w_enc = din("w_enc", (C, Di), bfl)
    b_enc = din("b_enc", (Di, 1))
    w_in_x = din("w_in_x", (Di, DQ), bfl)      # own-slice columns
    b_in_x = din("b_in_x", (DQ, 1))            # own slice
    w_in_z = din("w_in_z", (Di, DQ), bfl)
    b_in_z = din("b_in_z", (DQ, 1))
    w_cd = din("w_cd", (DQ, KC, DQ), bfl)      # diagonal conv weights per k
    conv_b = din("conv_b", (DQ, 1))            # own slice
    w_xp = din("w_xp", (DQ, 128), bfl)         # own rows, cols zero-padded per batch
    w_dt = din("w_dt", (128, DQ), bfl)  # rows zero-padded per batch
    b_dt = din("b_dt", (DQ, 1))
    a_sl = din("a_sl", (DQ, N))
    sel_bc = din("sel_bc", (128, 2 * N, 128), bfl)  # one-hot B/C row selectors
    d_col = din("d_col", (DQ, 1))
    w_mo = din("w_mo", (2 * Di, Di), bfl)      # rows of other batch group zeroed
    b_mo = din("b_mo", (Di, 1))
    w_out = din("w_out", (Di, C), bfl)
    b_out = din("b_out", (C, 1))
    g_col = din("g_col", (C, 1))
    bln_col = din("bln_col", (C, 1))

    res_out = nc.declare_dram_parameter("res", [C, 2 * TQ], fp32, isOutput=True)

    with tile.TileContext(nc) as tc:
        import contextlib
        with contextlib.ExitStack() as stack:
            wpool = stack.enter_context(tc.tile_pool(name="weights", bufs=1))
            cpool = stack.enter_context(tc.tile_pool(name="consts", bufs=1))
            dpool = stack.enter_context(tc.tile_pool(name="drambuf", bufs=1, space="DRAM"))

            ar_in_h = [dpool.tile([128, LH], bfl, name=f"arin{j}") for j in range(2)]
            ar_q = [dpool.tile([128, 2 * LC], bfl, name=f"arq{j}") for j in range(2)]
            ar_qo = [dpool.tile([128, 2 * LC], bfl, name=f"arqo{j}") for j in range(2)]
            ar_out_h = [dpool.tile([128, LH], bfl, name=f"arout{j}") for j in range(2)]
            ln_mu = [dpool.tile([1, TQ], bfl, name=f"lnmu{h}") for h in range(2)]
            ln_rs = [dpool.tile([1, TQ], fp32, name=f"lnrs{h}") for h in range(2)]
            a2a_in = [dpool.tile([2 * Di, TQ], bfl, name=f"a2ai{h}") for h in range(2)]
            a2a_out = [dpool.tile([2 * Di, TQ], bfl, name=f"a2ao{h}") for h in range(2)]

            def wload(ap, kt, m, name):
                t = wpool.tile([128, kt, m], bfl, tag=name, name=name)
                nc.sync.dma_start(out=t[:], in_=ap.ap().rearrange("(t k) m -> k t m", k=128))
                return t

            sw_dec_x = wload(w_dec_x, 2, Di, "w_dec_x")
            sw_enc = wload(w_enc, 2, Di, "w_enc")
            sw_dec_g = wload(w_dec_g, 2, Di, "w_dec_g")
            sw_in_x = wload(w_in_x, 4, DQ, "w_in_x")
            sw_in_z = wload(w_in_z, 4, DQ, "w_in_z")
            sw_xp = wpool.tile([128, 128], bfl)
            nc.sync.dma_start(out=sw_xp[:], in_=w_xp.ap())
            sw_mo = wload(w_mo, 8, Di, "w_mo")
            sw_out = wload(w_out, 4, C, "w_out")
            sw_dt = wpool.tile([128, DQ], bfl)
            nc.sync.dma_start(out=sw_dt[:], in_=w_dt.ap())
            sw_sel = wpool.tile([128, 2 * N, 128], bfl)
            nc.sync.dma_start(out=sw_sel[:], in_=sel_bc.ap())
            sw_cd = wpool.tile([128, KC, DQ], bfl)
            nc.sync.dma_start(out=sw_cd[:], in_=w_cd.ap())

            def cload(ap, nt, name, cols=1):
                if nt == 1:
                    t = cpool.tile([128, cols], fp32, tag=name, name=name)
                    nc.sync.dma_start(out=t[:], in_=ap.ap())
                else:
                    t = cpool.tile([128, nt, cols], fp32, tag=name, name=name)
                    nc.sync.dma_start(out=t[:], in_=ap.ap().rearrange("(t k) o -> k t o", k=128))
                return t

            sb_dec_x = cload(b_dec_x, 4, "b_dec_x")
            sb_dec_g = cload(b_dec_g, 4, "b_dec_g")
            sb_enc = cload(b_enc, 4, "b_enc")
            sb_in_x = cload(b_in_x, 1, "b_in_x")
            sb_in_z = cload(b_in_z, 1, "b_in_z")
            s_convb = cload(conv_b, 1, "conv_b")
            sb_dt = cload(b_dt, 1, "b_dt")
            s_a = cload(a_sl, 1, "a_sl", cols=N)
            s_d = cload(d_col, 1, "d_col")
            sb_mo = cload(b_mo, 4, "b_mo")
            sb_out = cload(b_out, 2, "b_out")
            s_g = cload(g_col, 2, "g_col")
            s_bln = cload(bln_col, 2, "bln_col")

            # batch-select scalar -> sync-engine register for cond DMAs
            bsel = nc.declare_dram_parameter("bsel", [1, 1], mybir.dt.int32,
                                             isOutput=False)
            # persistent tiles
            ppool = stack.enter_context(tc.tile_pool(name="persist", bufs=1))
            s_dt = ppool.tile([128, L], bfl)
            s_siluz = ppool.tile([128, L], bfl)
            s_uown = ppool.tile([128, L], bfl)
            s_sgate = ppool.tile([128, 4, 2 * TQ], bfl)
            s_decf = ppool.tile([128, 2, 2 * TQ], fp32)
            carry = ppool.tile([128, N], fp32)
            nc.vector.memset(carry[:], 0.0)
            ones = ppool.tile([128, 1], fp32)
            nc.vector.memset(ones[:], 1.0)
            ones_bf = ppool.tile([128, 1], bfl)
            nc.vector.memset(ones_bf[:], 1.0)
            eps = ppool.tile([1, 1], fp32)
            nc.vector.memset(eps[:], 1e-5)
            s_bsel = ppool.tile([1, 1], mybir.dt.int32)
            nc.sync.dma_start(out=s_bsel[:], in_=bsel.ap())
            breg = nc.sync.alloc_register("bsel_reg")
            nc.sync.reg_load(breg, s_bsel[0:1, 0:1])
            bsnap = nc.sync.snap(breg, min_val=0, max_val=1)
            zreg = nc.sync.alloc_register("zero_reg")
            nc.sync.reg_mov(zreg, 0)
            zsnap = nc.sync.snap(zreg, min_val=0, max_val=0)
            cond_b0 = bsnap != zsnap   # true on batch-0 cores (bsel=1)
            cond_b1 = bsnap == zsnap

            # rotating pools (shared across phases)
            fpool = stack.enter_context(tc.tile_pool(name="front", bufs=2))
            f1c = stack.enter_context(tc.tile_pool(name="small", bufs=2))
            psA = stack.enter_context(tc.tile_pool(name="psA", bufs=6, space="PSUM"))
            psts = stack.enter_context(tc.tile_pool(name="psts", bufs=1, space="PSUM"))

            # ---- pretail: decoder gate on own tokens (sigmoid table) ----
            nc.sync.dma_start(out=s_decf[:],
                              in_=dec_f32q.ap().rearrange("(t k) l -> k t l", k=128))
            with tc.tile_pool(name="pret", bufs=1) as prepool:
                s_decq = prepool.tile([128, 2, 2 * TQ], bfl)
                nc.gpsimd.tensor_copy(s_decq[:], s_decf[:])
                for lc in range(2):
                    ls = ts(lc, TQ)
                    for m in range(4):
                        ps_g = psA.tile([128, TQ], fp32, tag="mm", name="ps_g")
                        for t in range(2):
                            nc.tensor.matmul(ps_g[:], sw_dec_g[:, t, ts(m, 128)],
                                             s_decq[:, t, ls], start=(t == 0), stop=(t == 1))
                        nc.scalar.activation(s_sgate[:, m, ls], ps_g[:], AF.Sigmoid,
                                             bias=sb_dec_g[:, m, :])

            dec_r = dec_bf.ap().rearrange("(t k) l -> k t l", k=128)
            enc_r = enc_bf.ap().rearrange("(t k) l -> k t l", k=128)
            fstate = {}
            G8 = [[0, 1, 2, 3, 4, 5, 6, 7]]
            BROW = 32   # x_dbl rows: [0:32)=dt_in, [32:48)=B, [48:64)=C
            s_dtraw = ppool.tile([128, LH], bfl)   # staged softplus input (1 half)
            s_ex = ppool.tile([128, LH], bfl)      # softplus exp scratch

            def ph1_chunk(lc, on_pool=False):
                # combined = dec_x*sig(enc_p) + enc_p for one chunk
                ls = ts(lc, LC)
                s_dec = fpool.tile([128, 2, LC], bfl, tag="s_dec", name="s_dec")
                s_enc = fpool.tile([128, 2, LC], bfl, tag="s_enc", name="s_enc")
                nc.sync.dma_start(out=s_dec[:], in_=dec_r[:, :, ls])
                nc.sync.dma_start(out=s_enc[:], in_=enc_r[:, :, ls])
                comb = fpool.tile([128, 4, LC], bfl, tag="comb", name="comb")
                for m in range(4):
                    ps_dx = psA.tile([128, LC], fp32, tag="mm", name="ps_dx")
                    ps_ep = psA.tile([128, LC], fp32, tag="mm", name="ps_ep")
                    for t in range(2):
                        nc.tensor.matmul(ps_dx[:], sw_dec_x[:, t, ts(m, 128)],
                                         s_dec[:, t, :], start=(t == 0), stop=(t == 1))
                    for t in range(2):
                        nc.tensor.matmul(ps_ep[:], sw_enc[:, t, ts(m, 128)],
                                         s_enc[:, t, :], start=(t == 0), stop=(t == 1))
                    sg = f1c.tile([128, LC], bfl, tag="sg", name="sg")
                    nc.scalar.activation(sg[:], ps_ep[:], AF.Sigmoid,
                                         bias=sb_enc[:, m, :])
                    if on_pool:
                        # keep DVE free during the scan: materialize biased
                        # dx/ep via ACT (table-free), combine on Pool
                        dxs = f1c.tile([128, LC], bfl, tag="dxs", name="dxs")
                        nc.scalar.activation(dxs[:], ps_dx[:], AF.Identity,
                                             bias=sb_dec_x[:, m, :])
                        nc.scalar.activation(comb[:, m, :], ps_ep[:], AF.Identity,
                                             bias=sb_enc[:, m, :])
                        tm = f1c.tile([128, LC], bfl, tag="tm", name="tm")
                        nc.gpsimd.tensor_tensor(tm[:], dxs[:], sg[:], OP.mult)
                        nc.gpsimd.tensor_tensor(comb[:, m, :], comb[:, m, :], tm[:],
                                                OP.add)
                    else:
                        tm = f1c.tile([128, LC], bfl, tag="tm", name="tm")
                        nc.vector.scalar_tensor_tensor(tm[:], ps_dx[:],
                                                       sb_dec_x[:, m, :], sg[:],
                                                       OP.add, OP.mult)
                        nc.vector.scalar_tensor_tensor(comb[:, m, :], ps_ep[:],
                                                       sb_enc[:, m, :], tm[:],
                                                       OP.add, OP.add)
                return comb

            def front_rest_chunk(lc, comb, act_bias):
                # in_proj (own slice), conv (PE diag), padded x_proj partial,
                # stage partial x_dbl for the per-quarter AllReduce.
                ls = ts(lc, LC)
                half, i = lc // NLH, lc % NLH
                xm = fpool.tile([128, 3 + LC], bfl, tag="xm", name="xm")
                if lc == 0:
                    nc.gpsimd.memset(xm[:, 0:3], 0.0)
                else:
                    nc.scalar.activation(xm[:, 0:3],
                                         fstate["xm_prev"][:, LC:LC + 3],
                                         AF.Identity)
                ps_xm = psA.tile([128, LC], fp32, tag="mm", name="ps_xm")
                for t in range(4):
                    nc.tensor.matmul(ps_xm[:], sw_in_x[:, t, :],
                                     comb[:, t, :], start=(t == 0), stop=(t == 3))
                if act_bias:
                    nc.scalar.activation(xm[:, 3:3 + LC], ps_xm[:],
                                         AF.Identity, bias=sb_in_x[:, 0:1])
                else:
                    nc.vector.tensor_scalar(xm[:, 3:3 + LC], ps_xm[:],
                                            sb_in_x[:, 0:1], None, OP.add)
                ps_z = psA.tile([128, LC], fp32, tag="mm", name="ps_z")
                for t in range(4):
                    nc.tensor.matmul(ps_z[:], sw_in_z[:, t, :], comb[:, t, :],
                                     start=(t == 0), stop=(t == 3))
                nc.scalar.activation(s_siluz[:, ls], ps_z[:], AF.Silu,
                                     bias=sb_in_z[:, 0:1])
                # conv on PE: 4 shift-accumulated diagonal matmuls (own slice)
                ps_c = psA.tile([128, LC], fp32, tag="mm", name="ps_c")
                for k in range(KC):
                    nc.tensor.matmul(ps_c[:], sw_cd[:, k, :], xm[:, k:k + LC],
                                     start=(k == 0), stop=(k == KC - 1))
                nc.scalar.activation(s_uown[:, ls], ps_c[:], AF.Silu,
                                     bias=s_convb[:, 0:1])
                fstate["xm_prev"] = xm
                # padded x_proj partial: out rows [b*64, b*64+64) hold x_dbl
                ps_xd = psA.tile([128, LC], fp32, tag="mm", name="ps_xd")
                nc.tensor.matmul(ps_xd[:], sw_xp[:, :], s_uown[:, ls],
                                 start=True, stop=True)
                ar_st = f1c.tile([128, LC], bfl, tag="ar_st", name="ar_st")
                nc.scalar.activation(ar_st[:], ps_xd[:], AF.Copy)
                if half == 0:
                    # per-quarter AllReduce so the reduce overlaps the front
                    nc.sync.dma_start(out=ar_q[i // 2][:, ts(i % 2, LC)],
                                      in_=ar_st[:])
                    if i % 2 == 1:
                        nc.gpsimd.collective_compute(
                            "AllReduce", OP.add, replica_groups=G8,
                            ins=[ar_q[i // 2][:, :]], outs=[ar_qo[i // 2][:, :]],
                        )
                else:
                    nc.sync.dma_start(out=ar_in_h[half][:, ts(i, LC)], in_=ar_st[:])
                    if i == NLH - 1:
                        nc.gpsimd.collective_compute(
                            "AllReduce", OP.add, replica_groups=G8,
                            ins=[ar_in_h[half][:, :]], outs=[ar_out_h[half][:, :]],
                        )

            def softplus_half(half):
                # dt matmul over all 128 AllReduced rows (other batch rows are
                # zeroed in sw_dt), then softplus(x) = ln(1 + exp(x + dt_b))
                hs = ts(half, LH)
                dtin = f1c.tile([128, LH], bfl, tag="dtin", name="dtin")
                if half == 0:
                    for j in range(2):
                        nc.sync.dma_start(out=dtin[:, ts(j, 2 * LC)],
                                          in_=ar_qo[j][:, :])
                else:
                    nc.sync.dma_start(out=dtin[:], in_=ar_out_h[half][:, :])
                fstate[f"xdbl{half}"] = dtin
                for fc in range(4):
                    ps_dt = psA.tile([128, LC], fp32, tag="mm", name="ps_dt")
                    nc.tensor.matmul(ps_dt[:], sw_dt[:, :], dtin[:, ts(fc, LC)],
                                     start=True, stop=True)
                    nc.scalar.activation(s_dtraw[:, ts(fc, LC)], ps_dt[:], AF.Copy)
                nc.scalar.activation(s_ex[:], s_dtraw[:], AF.Exp, bias=sb_dt[:, 0:1])
                nc.scalar.activation(s_dt[:, hs], s_ex[:], AF.Ln, bias=1.0)

            def front(half):
                for i in range(NLH):
                    lc = half * NLH + i
                    comb = ph1_chunk(lc)
                    front_rest_chunk(lc, comb, act_bias=False)

            spool = stack.enter_context(tc.tile_pool(name="scan", bufs=1))
            sbc = stack.enter_context(tc.tile_pool(name="scanbc", bufs=2))
            sy = stack.enter_context(tc.tile_pool(name="scany", bufs=1))

            def scan(half, fillers=None):
                fillers = fillers or {}
                hs = ts(half, LH)
                s_dtu = sy.tile([128, LH], bfl, tag="dtu", name="s_dtu")
                nc.vector.tensor_tensor(s_dtu[:], s_dt[:, hs], s_uown[:, hs], OP.mult)
                s_uD = sy.tile([128, LH], bfl, tag="uD", name="s_uD")
                nc.vector.tensor_scalar(s_uD[:], s_uown[:, hs], s_d[:, 0:1], None,
                                        OP.mult)
                ysum = sy.tile([128, LH], bfl, tag="ysum", name="ysum")
                xdbl = fstate[f"xdbl{half}"]
                for n in range(N):
                    w = n % 2
                    bc2 = sbc.tile([128, 2, LH], bfl, tag="bc2", name="bc2")
                    for s in range(2):          # 0: B row, 1: C row
                        for fc in range(4):
                            ps_bc = psA.tile([128, LC], fp32, tag="mm",
                                             name="ps_bc")
                            nc.tensor.matmul(ps_bc[:], sw_sel[:, s * N + n, :],
                                             xdbl[:, ts(fc, LC)],
                                             start=True, stop=True)
                            nc.scalar.activation(bc2[:, s, ts(fc, LC)], ps_bc[:],
                                                 AF.Copy)
                    bbc = bc2[:, 0, :]
                    cbc = bc2[:, 1, :]
                    a = spool.tile([128, LH], bfl, tag=f"a{n % 3}", name="a")
                    nc.scalar.activation(a[:], s_dt[:, hs], AF.Exp,
                                         scale=s_a[:, n:n + 1])
                    bt = spool.tile([128, LH], bfl, tag=f"b{w}", name=f"b{w}")
                    nc.vector.tensor_tensor(bt[:], s_dtu[:], bbc, OP.mult)
                    h = spool.tile([128, LH], bfl, tag=f"h{w}", name=f"h{w}")
                    nc.vector.tensor_tensor_scan(h[:], a[:], bt[:],
                                                 carry[:, n:n + 1], OP.mult, OP.add)
                    if half == 0:
                        nc.vector.tensor_copy(carry[:, n:n + 1], h[:, LH - 1:LH])
                    nc.vector.tensor_tensor(h[:], h[:], cbc, OP.mult)
                    if n == 0:
                        nc.vector.tensor_tensor(ysum[:], h[:], s_uD[:], OP.add)
                    else:
                        nc.vector.tensor_tensor(ysum[:], ysum[:], h[:], OP.add)
                    if n in fillers:
                        fillers[n]()
                y_h = sy.tile([128, LH], bfl, tag="y", name="y_h")
                nc.vector.tensor_tensor(y_h[:], ysum[:], s_siluz[:, hs], OP.mult)
                for j in range(8):
                    nc.sync.dma_start(out=a2a_in[half][j * 128:(j + 1) * 128, :],
                                      in_=y_h[:, ts(j % 4, TQ)])
                nc.gpsimd.collective_compute(
                    "AllToAll", mybir.AluOpType.bypass, replica_groups=G8,
                    ins=[a2a_in[half][:, :]], outs=[a2a_out[half][:, :]],
                )

            tpool = stack.enter_context(tc.tile_pool(name="tail", bufs=1))

            tstate = {}

            def tail_main(half):
                hs = ts(half, TQ)
                s_yall = tpool.tile([128, 8, TQ], bfl, tag="yall", name="s_yall")
                a2a_r = a2a_out[half][:, :].rearrange("(t k) l -> k t l", k=128)
                nc.sync.dma_start(out=s_yall[:], in_=a2a_r)
                s_gated = tpool.tile([128, 4, TQ], bfl, tag="gated", name="s_gated")
                for m in range(4):
                    ps_mo = psA.tile([128, TQ], fp32, tag="mm", name="ps_mo")
                    for t in range(8):
                        nc.tensor.matmul(ps_mo[:], sw_mo[:, t, ts(m, 128)],
                                         s_yall[:, t, :], start=(t == 0), stop=(t == 7))
                    spr = f1c.tile([128, TQ], bfl, tag="spr", name="spr")
                    nc.scalar.activation(spr[:], ps_mo[:], AF.Identity,
                                         bias=sb_mo[:, m, :])
                    nc.gpsimd.tensor_tensor(s_gated[:, m, :], spr[:],
                                            s_sgate[:, m, hs], OP.mult)
                s_res = tpool.tile([128, 2, TQ], bfl, tag=f"res{half}", name="s_res")
                ps_sum = psts.tile([1, TQ], fp32, tag="ps_sum", name="ps_sum")
                ps_sq = psts.tile([1, TQ], fp32, tag="ps_sq", name="ps_sq")
                for m in range(2):
                    ps_o = psA.tile([128, TQ], fp32, tag="mm", name="ps_o")
                    for t in range(4):
                        nc.tensor.matmul(ps_o[:], sw_out[:, t, ts(m, 128)],
                                         s_gated[:, t, :], start=(t == 0), stop=(t == 3))
                    otmp = f1c.tile([128, TQ], bfl, tag="otmp", name="otmp")
                    nc.scalar.activation(otmp[:], ps_o[:], AF.Identity,
                                         bias=sb_out[:, m, :])
                    nc.gpsimd.tensor_tensor(s_res[:, m, :], otmp[:],
                                            s_decf[:, m, hs], OP.add)
                    res2 = f1c.tile([128, TQ], bfl, tag="res2", name="res2")
                    nc.scalar.activation(res2[:], s_res[:, m, :], AF.Square)
                    nc.tensor.matmul(ps_sum[:], ones_bf[:], s_res[:, m, :],
                                     start=(m == 0), stop=(m == 1))
                    nc.tensor.matmul(ps_sq[:], ones_bf[:], res2[:],
                                     start=(m == 0), stop=(m == 1))
                mu = tpool.tile([1, TQ], bfl, tag="mu", name="mu")
                nc.scalar.activation(mu[:], ps_sum[:], AF.Copy, scale=1.0 / C)
                musq = tpool.tile([1, TQ], bfl, tag="musq", name="musq")
                nc.scalar.activation(musq[:], mu[:], AF.Square)
                sq_sb = tpool.tile([1, TQ], bfl, tag="sq_sb", name="sq_sb")
                nc.scalar.activation(sq_sb[:], ps_sq[:], AF.Copy, scale=1.0 / C)
                var = tpool.tile([1, TQ], bfl, tag="var", name="var")
                nc.gpsimd.tensor_tensor(var[:], sq_sb[:], musq[:], OP.subtract)
                sd = tpool.tile([1, TQ], fp32, tag=f"sd{half}", name="sd")
                nc.scalar.activation(sd[:], var[:], AF.Sqrt, bias=eps[:, 0:1])
                nc.sync.dma_start(out=ln_mu[half][0:1, :], in_=mu[:])
                tstate[half] = (s_res, sd)

            def tail_fin(half):
                hs = ts(half, TQ)
                s_res, sd = tstate[half]
                rstd = tpool.tile([1, TQ], fp32, tag="rstd", name="rstd")
                nc.vector.reciprocal(rstd[:], sd[:])
                nc.sync.dma_start(out=ln_rs[half][0:1, :], in_=rstd[:])
                mu_bc = tpool.tile([128, TQ], bfl, tag="mu_bc", name="mu_bc")
                rs_bc = tpool.tile([128, TQ], fp32, tag="rs_bc", name="rs_bc")
                lnm = ln_mu[half][0:1, 0:1]
                lnr = ln_rs[half][0:1, 0:1]
                nc.sync.dma_start(out=mu_bc[:], in_=bass.AP(
                    tensor=lnm.tensor, offset=lnm.offset, ap=[[0, 128], [1, TQ]]))
                nc.sync.dma_start(out=rs_bc[:], in_=bass.AP(
                    tensor=lnr.tensor, offset=lnr.offset, ap=[[0, 128], [1, TQ]]))
                for m in range(2):
                    nc.gpsimd.tensor_tensor(s_res[:, m, :], s_res[:, m, :], mu_bc[:],
                                            OP.subtract)
                    nc.gpsimd.tensor_tensor(s_res[:, m, :], s_res[:, m, :], rs_bc[:],
                                            OP.mult)
                    t1 = tpool.tile([128, TQ], fp32, tag="t1", name="t1")
                    nc.scalar.activation(t1[:], s_res[:, m, :], AF.Identity,
                                         scale=s_g[:, m, :], bias=s_bln[:, m, :])
                    nc.sync.dma_start(
                        out=res_out.ap().rearrange("(t k) l -> k t l", k=128)[:, m, hs],
                        in_=t1[:])

            # ---- emission order ----
            front(0)
            front(1)
            softplus_half(0)
            scan(0, {10: lambda: softplus_half(1)})
            scan(1, {14: lambda: tail_main(0)})
            tail_fin(0)
            tail_main(1)
            tail_fin(1)

    nc.compile()
    return nc


def _in_maps(inp):
    A = -np.exp(inp["A_log"].astype(np.float32))
    dec_T = inp["decoder_feat"].reshape(B, C, L)
    enc_T = inp["encoder_feat"].reshape(B, C, L)
    dec_T_bf = dec_T.astype(bf16)
    enc_T_bf = enc_T.astype(bf16)

    def col(x):
        return np.ascontiguousarray(np.asarray(x, np.float32).reshape(-1, 1))

    common = {
        "w_dec_x": np.ascontiguousarray(inp["dec_w"][:, :Di].astype(bf16)),
        "w_dec_g": np.ascontiguousarray(inp["dec_w"][:, Di:].astype(bf16)),
        "b_dec_x": col(inp["dec_b"][:Di]),
        "b_dec_g": col(inp["dec_b"][Di:]),
        "w_enc": inp["enc_w"].astype(bf16),
        "b_enc": col(inp["enc_b"]),
        "b_mo": col(inp["m_out_b"]),
        "w_out": inp["out_w"].astype(bf16),
        "b_out": col(inp["out_b"]),
        "g_col": col(inp["ln_g"]),
        "bln_col": col(inp["ln_b"]),
    }

    in_maps = []
    for c in range(NCORES):
        b, q = c // 4, c % 4
        ds = slice(q * DQ, (q + 1) * DQ)
        m = dict(common)
        m["dec_bf"] = dec_T_bf[b]
        m["enc_bf"] = enc_T_bf[b]
        tok = np.r_[np.arange(q * TQ, (q + 1) * TQ),
                    np.arange(LH + q * TQ, LH + (q + 1) * TQ)]
        m["dec_f32q"] = np.ascontiguousarray(dec_T[b][:, tok].astype(np.float32))
        m["w_in_x"] = np.ascontiguousarray(inp["in_w"][:, :Di][:, ds].astype(bf16))
        m["b_in_x"] = col(inp["in_b"][:Di][ds])
        m["w_in_z"] = np.ascontiguousarray(
            inp["in_w"][:, Di + q * DQ:Di + (q + 1) * DQ].astype(bf16))
        m["b_in_z"] = col(inp["in_b"][Di + q * DQ:Di + (q + 1) * DQ])
        cw = inp["conv_w"][ds, 0, :].astype(np.float32)     # (DQ, KC) own slice
        wcd = np.zeros((DQ, KC, DQ), np.float32)
        idx = np.arange(DQ)
        for k in range(KC):
            wcd[idx, k, idx] = cw[idx, k]
        m["w_cd"] = wcd.astype(bf16)
        m["conv_b"] = col(inp["conv_b"][ds])
        # padded x_proj: own 128 rows -> 128 out cols, my batch block only
        wxp = np.zeros((DQ, 128), np.float32)
        wxp[:, b * 64:b * 64 + 2 * R] = inp["x_proj_w"][ds, :]
        m["w_xp"] = wxp.astype(bf16)
        # padded dt weights: contraction over all 128 AR rows, other rows 0
        wdt = np.zeros((128, DQ), np.float32)
        wdt[b * 64:b * 64 + R, :] = inp["dt_w"][:, ds]
        m["w_dt"] = wdt.astype(bf16)
        m["b_dt"] = col(inp["dt_b"][ds])
        m["bsel"] = np.array([[1 - b]], np.int32)
        sel = np.zeros((128, 2 * N, 128), np.float32)
        for n in range(N):
            sel[b * 64 + R + n, n, :] = 1.0          # B row
            sel[b * 64 + R + N + n, N + n, :] = 1.0  # C row
        m["sel_bc"] = sel.astype(bf16)
        wmo8 = np.zeros((2 * Di, Di), np.float32)
        for r in range(8):
            if r // 4 == b:
                rq = r % 4
                wmo8[r * DQ:(r + 1) * DQ] = inp["m_out_w"][rq * DQ:(rq + 1) * DQ]
        m["w_mo"] = wmo8.astype(bf16)
        m["a_sl"] = np.ascontiguousarray(A[ds])
        m["d_col"] = col(inp["D_param"][ds])
        in_maps.append(m)
    return in_maps


def kernel(**inputs):
    from concourse.bass_utils import run_bass_kernel_spmd

    inp = {k: np.asarray(v) for k, v in inputs.items()}
    if "nc" not in _cache:
        _cache["nc"] = _build()
    res = run_bass_kernel_spmd(_cache["nc"], _in_maps(inp), list(range(NCORES)))
    out = np.zeros((B, C, L), np.float32)
    for c in range(NCORES):
        b, q = c // 4, c % 4
        r = res.results[c]["res"]
        out[b][:, q * TQ:(q + 1) * TQ] = r[:, 0:TQ]
        out[b][:, LH + q * TQ:LH + (q + 1) * TQ] = r[:, TQ:2 * TQ]
    return out.reshape(B, C, Hh, Ww)


def run_traced(inp):
    from concourse.bass_utils import run_bass_kernel_spmd

    if "nc" not in _cache:
        _cache["nc"] = _build()
    return run_bass_kernel_spmd(_cache["nc"], _in_maps(inp), list(range(NCORES)),
                                trace=True)
